# revision 9
# baseline (speedup 1.0000x reference)
"""Causal multi-head attention (B=2, L=2048, D=1024, H=16) on 8 trn2 cores.

Sharding: DP on batch (2) x TP on heads (4 groups of 4 heads) = 8 cores.
Each core computes, for its (batch b, head-group g):
  - qT/kT = wqk_g^T @ x_b^T            [512, L]   (head dims on partitions)
  - V     = x_b @ wv_g (+ ones cols)   [L, 4*65]  (natural layout, per-head ones
                                                   column so the PV matmul also
                                                   produces softmax denominators)
  - S^T   = K Q^T per (k-block, q-tile), causal-trimmed, both heads of a
            pair row-packed into one concurrent PE pass; additive causal mask
            (-240 pre-scale) accumulated into PSUM by a small mask @ I matmul
            on the diagonal squares; ONE exp per k-block on ACT (3-D AP
            merging both heads' trimmed spans)
  - out^T = V_ext^T @ E^T accumulated over k-blocks  -> PSUM
            (partition 64 resp. 32 holds the softmax denominator r)
  - attn^T = out^T * (1/r); 1/r via DVE reciprocal_approx_fast on the r rows;
            broadcast across partitions via a step-0-free-dim SBUF->SBUF DMA;
            DVE multiply during PSUM eviction
  - y_part = attn @ w_out[rows of g]   [L, 1024]  (row-parallel out-proj)
Host gathers: y_b = sum_g y_part + (b_qkv_v @ w_out + b_out).

Engine-balance: exp is the ACT bottleneck (~80us); everything else is kept
off ACT (mask on PE, reciprocal on DVE, half the y evictions on DVE).
Emission is software-pipelined: all pair-0 attention runs right after the
q/k projections it needs (m=0,2), with the pair-1 projections (m=1,3) and
the out-projection tiles popped into the PE stream between attention units
so the PE never idles while ACT grinds exp.
"""

import sys
from collections import deque
from contextlib import ExitStack

if "/opt/trn_rl_repo" not in sys.path:
    sys.path.insert(0, "/opt/trn_rl_repo")

import ml_dtypes
import numpy as np

import concourse.bass as bass
import concourse.mybir as mybir
import concourse.tile as tile
from concourse import bacc
from concourse.bass import ts
from concourse.bass_utils import run_bass_kernel_spmd

F32 = mybir.dt.float32
BF16 = mybir.dt.bfloat16
AF = mybir.ActivationFunctionType
OP = mybir.AluOpType

B, D, H = 2, 1024, 16
HD = 64           # head dim
NH = 4            # heads per core
GD = NH * HD      # 256 head dims per core
P = 128
QTW = 512         # q-tile width
VSTR = 193        # per-pair stride in the v tile: [V0(64)|1] + [z32|1|z31|V1(64)]
VW = 2 * VSTR     # v tile width (2 pairs)
MASKV = -240.0    # additive causal mask, pre-exp-scale (0.125) -> -30


def bcast_ap(row_ap, n_part):
    """[1, N] SBUF AP -> (1, n_part, N) AP replicating the row (step-0 free
    dim), for DMA partition-broadcast."""
    from concourse.ap import AP

    dims = list(row_ap.ap)
    assert dims[0][1] == 1 and len(dims) == 2, dims
    return AP(row_ap.tensor, row_ap.offset,
              [list(dims[0]), [0, n_part], list(dims[1])])


def build_nc(L=2048):
    """Build the per-core Bass program. Same program for all 8 cores (SPMD)."""
    DK = D // P       # 8 contraction chunks
    LT = L // P       # l-tiles
    QT = L // QTW     # q-tiles
    QB = QTW // P     # k-blocks per q-tile (4)

    nc = bacc.Bacc("TRN2", target_bir_lowering=False, debug=False, num_devices=8)

    xT = nc.dram_tensor("xT", [D, L], BF16, kind="ExternalInput").ap()
    wqk = nc.dram_tensor("wqk", [D, 2 * GD], BF16, kind="ExternalInput").ap()
    wv = nc.dram_tensor("wv", [D, GD], BF16, kind="ExternalInput").ap()
    wo = nc.dram_tensor("wo", [GD, D], BF16, kind="ExternalInput").ap()
    bqk = nc.dram_tensor("bqk", [2 * GD, 1], F32, kind="ExternalInput").ap()
    maskT = nc.dram_tensor("maskT", [P, P], BF16, kind="ExternalInput").ap()
    ident = nc.dram_tensor("ident", [P, P], BF16, kind="ExternalInput").ap()
    # ones/zeros filler for the V slots: [1, 0*32, 1, 0*31] per partition
    vpat = nc.dram_tensor("vpat", [P, 65], BF16, kind="ExternalInput").ap()
    y = nc.dram_tensor("y", [L, D], F32, kind="ExternalOutput").ap()

    with tile.TileContext(nc) as tc, ExitStack() as stk:
        # ---------- persistent pools ----------
        const = stk.enter_context(tc.tile_pool(name="const", bufs=1))
        qk_pool = stk.enter_context(tc.tile_pool(name="qk", bufs=1))
        v_pool = stk.enter_context(tc.tile_pool(name="v", bufs=1))
        attn_pool = stk.enter_context(tc.tile_pool(name="attn", bufs=1))
        wo_pool = stk.enter_context(tc.tile_pool(name="wo", bufs=1))
        xt_pool = stk.enter_context(tc.tile_pool(name="xt", bufs=1))
        wi_pool = stk.enter_context(tc.tile_pool(name="wi", bufs=1))

        xt_sb = [xt_pool.tile([P, L], BF16, tag=f"xt{k}", name=f"xt_sb{k}") for k in range(DK)]
        wqk_sb = [wi_pool.tile([P, 2 * GD], BF16, tag=f"wqk{k}", name=f"wqk_sb{k}") for k in range(DK)]
        wv_sb = [wi_pool.tile([P, GD], BF16, tag=f"wv{k}", name=f"wv_sb{k}") for k in range(DK)]
        for k in range(DK):
            nc.sync.dma_start(wv_sb[k][:], wv[ts(k, P)])
            nc.sync.dma_start(xt_sb[k][:], xT[ts(k, P)])
        for k in range(DK):
            nc.sync.dma_start(wqk_sb[k][:], wqk[ts(k, P)])

        bqk_sb = const.tile([P, 4], F32, tag="bqk", name="bqk_sb")
        for m in range(4):
            nc.sync.dma_start(bqk_sb[:, m : m + 1], bqk[ts(m, P)])
        maskT_sb = const.tile([P, P], BF16, tag="maskT", name="maskT_sb")
        nc.sync.dma_start(maskT_sb[:], maskT)
        ident_sb = const.tile([P, P], BF16, tag="ident", name="ident_sb")
        nc.sync.dma_start(ident_sb[:], ident)

        # m-tile 0,1 = qT (head pairs 01, 23); 2,3 = kT
        qk_sb = [qk_pool.tile([P, L], BF16, tag=f"qk{m}", name=f"qk_sb{m}") for m in range(4)]
        v_sb = [v_pool.tile([P, VW], BF16, tag=f"v{t}", name=f"v_sb{t}") for t in range(LT)]
        attn_sb = [attn_pool.tile([P, L], BF16, tag=f"attn{p}", name=f"attn_sb{p}") for p in range(2)]
        wo_sb = [wo_pool.tile([P, D], BF16, tag=f"wo{c}", name=f"wo_sb{c}") for c in range(2)]
        for c in range(2):
            nc.sync.dma_start(wo_sb[c][:], wo[ts(c, P)])
        for lt in range(LT):
            vv = v_sb[lt][:, 0:VW].rearrange("p (a c) -> p a c", a=2, c=VSTR)
            for a in range(2):
                nc.sync.dma_start(vv[:, a, 64:129], vpat)

        # ---------- phase A: V projection (k-outer, 2 passes of 8 l-tiles) ----
        # V natural: [L, 256] = x @ wv, packed into per-head [V|ones] slots.
        # k-outer streams against the xT DMA arrival order.
        with tc.tile_pool(name="psv", bufs=1, space="PSUM") as psv:
            for half in range(2):
                lts = list(range(8 * half, 8 * half + 8))
                pvs = [psv.tile([P, GD], F32, tag=f"psv{i}", name=f"ps_v{i}")
                       for i in range(8)]
                for k in range(DK):
                    for i, lt in enumerate(lts):
                        nc.tensor.matmul(
                            pvs[i][:],
                            xt_sb[k][:, ts(lt, P)],
                            wv_sb[k][:],
                            start=(k == 0),
                            stop=(k == DK - 1),
                        )
                for i, lt in enumerate(lts):
                    vv = v_sb[lt][:, 0:VW].rearrange("p (a c) -> p a c", a=2, c=VSTR)
                    pv = pvs[i][:].rearrange("p (a c) -> p a c", a=2, c=2 * HD)
                    nc.vector.tensor_copy(vv[:, :, 0:64], pv[:, :, 0:64])      # heads 0,2
                    nc.vector.tensor_copy(vv[:, :, 129:193], pv[:, :, 64:128])  # heads 1,3

        # ---------- phase B helpers: q/k projections ----------
        # qT/kT: [512, L] = wqk^T @ xT, bias added during PSUM eviction.
        def qk_tile(m, ps_list, ns):
            """Emit k-loop matmuls for m-tile `m` over n-slices `ns` into the
            given psum tiles, then evict with bias-add."""
            for k in range(DK):
                for i, n in enumerate(ns):
                    nc.tensor.matmul(
                        ps_list[i][:],
                        wqk_sb[k][:, ts(m, P)],
                        xt_sb[k][:, ts(n, QTW)],
                        start=(k == 0),
                        stop=(k == DK - 1),
                    )
            for i, n in enumerate(ns):
                nc.vector.tensor_scalar(
                    out=qk_sb[m][:, ts(n, QTW)],
                    in0=ps_list[i][:],
                    scalar1=bqk_sb[:, m : m + 1],
                    scalar2=None,
                    op0=OP.add,
                )

        NT = L // QTW
        with tc.tile_pool(name="psp", bufs=1, space="PSUM") as psp:
            for m in (0, 2):
                ps4 = [psp.tile([P, QTW], F32, tag=f"psp{n}", name=f"ps_p{n}")
                       for n in range(NT)]
                qk_tile(m, ps4, list(range(NT)))

        # ---------- phase C: attention + out-proj, pipelined emission --------
        with (
            tc.tile_pool(name="e", bufs=4) as e_pool,
            tc.tile_pool(name="rinv", bufs=2) as r_pool,
            tc.tile_pool(name="bc", bufs=2) as bc_pool,
            tc.tile_pool(name="ysb", bufs=4) as y_pool,
            tc.tile_pool(name="pss", bufs=2, space="PSUM") as pss,
            tc.tile_pool(name="pso", bufs=1, space="PSUM") as pso,
        ):
            SKEW = 3
            popq = deque()

            def attn_pair(qt, pair, pop_every=0):
                """Attention for (q-tile qt, head-pair pair). Pops at most one
                queued filler closure every `pop_every` units (0 = never)."""
                q_t = qk_sb[pair]
                k_t = qk_sb[2 + pair]
                out_ps = [
                    pso.tile([P, QTW], F32, tag=f"pso{h}", name=f"ps_o{h}")
                    for h in range(2)
                ]
                nblk = QB * qt + QB     # k-blocks for this q-tile

                def front(j):
                    """Row-packed scores (+ additive mask) + exp for k-block j.
                    Returns a closure emitting the two PV matmuls."""
                    sp = pss.tile([P, 2 * QTW], F32, tag="pss", name="ps_s")
                    e_t = e_pool.tile([P, 2 * QTW], BF16, tag="e", name="e_t")
                    diag = j >= QB * qt
                    da = (j - QB * qt) * P if diag else 0
                    for hl in range(2):
                        hb = 64 * hl
                        nc.tensor.matmul(
                            sp[:, hl * QTW + da : (hl + 1) * QTW],
                            k_t[hb : hb + 64, ts(j, P)],
                            q_t[hb : hb + 64,
                                qt * QTW + da : (qt + 1) * QTW],
                            start=True, stop=True)
                    if diag:
                        # additive causal mask on the diagonal square:
                        # sp[:, da:da+P] += maskT.T @ I  (strict upper = -240)
                        for hl in range(2):
                            nc.tensor.matmul(
                                sp[:, hl * QTW + da : hl * QTW + da + P],
                                maskT_sb[:],
                                ident_sb[:],
                                start=False, stop=True,
                                skip_group_check=True)
                    if da == 0:
                        nc.scalar.activation(e_t[:], sp[:], AF.Exp, scale=0.125)
                    else:
                        spv = sp[:].rearrange("p (h c) -> p h c", h=2, c=QTW)
                        etv = e_t[:].rearrange("p (h c) -> p h c", h=2, c=QTW)
                        nc.scalar.activation(etv[:, :, da:QTW], spv[:, :, da:QTW],
                                             AF.Exp, scale=0.125)

                    def emit_pv(j=j, da=da, e_t=e_t):
                        for hl in range(2):
                            mm_pv(nc, out_ps[hl], hl, da,
                                  vext(v_sb[j], pair, hl),
                                  e_t[:, hl * QTW + da : (hl + 1) * QTW],
                                  start=(j == 0), stop=(j == nblk - 1))
                    return emit_pv

                pend = []
                for j in range(nblk):
                    pend.append(front(j))
                    if pop_every and popq and j % pop_every == (pop_every - 1):
                        popq.popleft()()
                    if j >= SKEW:
                        pend[j - SKEW]()
                for j in range(max(0, nblk - SKEW), nblk):
                    pend[j]()

                # normalize: 1/r on DVE (approx, ~18 bits), broadcast via DMA,
                # multiply during PSUM->SBUF eviction on DVE
                rinv = r_pool.tile([P, 2 * QTW], F32, tag="rinv", name="rinv_t")
                bc = bc_pool.tile([P, QTW], F32, tag="bc", name="bc_t")
                nc.scalar.activation(rinv[64:65, 0:QTW], out_ps[0][64:65, :],
                                     AF.Ln)
                nc.scalar.activation(rinv[32:33, 0:QTW], out_ps[1][32:33, :],
                                     AF.Ln)
                nc.scalar.activation(rinv[:, 0:QTW], rinv[:, 0:QTW], AF.Exp,
                                     scale=-1.0)
                nc.sync.dma_start(bc[0:64, :], bcast_ap(rinv[64:65, 0:QTW], 64))
                nc.sync.dma_start(bc[64:P, :], bcast_ap(rinv[32:33, 0:QTW], 64))
                nc.vector.tensor_tensor(
                    out=attn_sb[pair][0:64, ts(qt, QTW)],
                    in0=out_ps[0][0:64, :], in1=bc[0:64, :], op=OP.mult)
                nc.vector.tensor_tensor(
                    out=attn_sb[pair][64:P, ts(qt, QTW)],
                    in0=out_ps[1][64:P, :], in1=bc[64:P, :], op=OP.mult)

            # -- pair-0 attention for all q-tiles, with the m=1,3 projections
            #    popped in between units --
            with tc.tile_pool(name="psq", bufs=1, space="PSUM") as psq:
                attn_pair(0, 0)

                # queue the m=1,3 q/k projections as 8 half-k closures
                shared = {}
                for m in (1, 3):
                    for np_ in range(2):
                        ns = [2 * np_, 2 * np_ + 1]

                        def qk_a(m=m, np_=np_, ns=ns):
                            ps2 = [psq.tile([P, QTW], F32, tag=f"psq{i}",
                                            name=f"ps_q{i}")
                                   for i in range(2)]
                            shared[(m, np_)] = ps2
                            for k in range(DK // 2):
                                for i, n in enumerate(ns):
                                    nc.tensor.matmul(
                                        ps2[i][:],
                                        wqk_sb[k][:, ts(m, P)],
                                        xt_sb[k][:, ts(n, QTW)],
                                        start=(k == 0), stop=False)

                        def qk_b(m=m, np_=np_, ns=ns):
                            ps2 = shared[(m, np_)]
                            for k in range(DK // 2, DK):
                                for i, n in enumerate(ns):
                                    nc.tensor.matmul(
                                        ps2[i][:],
                                        wqk_sb[k][:, ts(m, P)],
                                        xt_sb[k][:, ts(n, QTW)],
                                        start=False, stop=(k == DK - 1))
                            for i, n in enumerate(ns):
                                nc.vector.tensor_scalar(
                                    out=qk_sb[m][:, ts(n, QTW)],
                                    in0=ps2[i][:],
                                    scalar1=bqk_sb[:, m : m + 1],
                                    scalar2=None,
                                    op0=OP.add,
                                )
                        popq.append(qk_a)
                        popq.append(qk_b)

                attn_pair(1, 0, pop_every=2)
                attn_pair(2, 0, pop_every=2)
                attn_pair(3, 0, pop_every=2)
                while popq:
                    popq.popleft()()

            # -- pair-1 attention + out-projection tiles popped in between --
            with tc.tile_pool(name="psy", bufs=2, space="PSUM") as psy:
                def push_op(qt):
                    for lt in range(4 * qt, 4 * qt + 4):
                        for nh in range(2):
                            def op_tile(lt=lt, nh=nh):
                                ps = psy.tile([P, QTW], F32, tag="psy",
                                              name="ps_y")
                                for c in range(2):
                                    nc.tensor.matmul(
                                        ps[:],
                                        attn_sb[c][:, ts(lt, P)],
                                        wo_sb[c][:, ts(nh, QTW)],
                                        start=(c == 0),
                                        stop=(c == 1),
                                    )
                                yt = y_pool.tile([P, QTW], F32, tag="y",
                                                 name="y_t")
                                if nh == 0:
                                    nc.scalar.copy(yt[:], ps[:])
                                else:
                                    nc.vector.tensor_copy(yt[:], ps[:])
                                nc.sync.dma_start(y[ts(lt, P), ts(nh, QTW)],
                                                  yt[:])
                            popq.append(op_tile)

                attn_pair(0, 1)
                push_op(0)
                attn_pair(1, 1, pop_every=2)
                push_op(1)
                attn_pair(2, 1, pop_every=1)
                push_op(2)
                attn_pair(3, 1, pop_every=1)
                push_op(3)
                while popq:
                    popq.popleft()()

    nc.compile()
    return nc


def vext(vt, pair, hl):
    """lhsT slice of the extended-V tile for (pair, local head hl)."""
    base = VSTR * pair
    if hl == 0:
        return vt[:, base : base + 65]          # M=65: V at 0-63, r at 64
    return vt[:, base + 65 : base + VSTR]       # M=128: ones@32, V at 64-127


def mm_pv(nc, out_ps, hl, c0, lhsT, rhs, start, stop):
    if hl == 0:
        out = out_ps[0:65, c0:QTW]
    else:
        out = out_ps[:, c0:QTW]
    nc.tensor.matmul(out, lhsT, rhs, start=start, stop=stop)


def make_maskT():
    # additive mask: want  (maskT.T @ I)[k, c] = MASKV if k > c else 0
    # => maskT[c, k] = MASKV for k > c: strict upper triangle.
    m = np.zeros((P, P), np.float32)
    m[np.arange(P)[:, None] < np.arange(P)[None, :]] = MASKV
    return m.astype(ml_dtypes.bfloat16)


def make_vpat():
    pat = np.zeros((P, 65), ml_dtypes.bfloat16)
    pat[:, 0] = 1.0   # even-head ones col (tile col 64): r -> partition 64
    pat[:, 33] = 1.0  # odd-head ones col (tile col 97): r -> partition 32
    return pat


def shard_inputs(x, w_qkv, b_qkv, w_out, L=2048):
    """Host-side sharding: core c = (batch c//4, head-group c%4)."""
    x = np.asarray(x, np.float32)
    w_qkv = np.asarray(w_qkv, np.float32)
    b_qkv = np.asarray(b_qkv, np.float32)
    w_out = np.asarray(w_out, np.float32)
    ident = np.eye(P, dtype=ml_dtypes.bfloat16)
    maskT = make_maskT()
    xTs = [np.ascontiguousarray(x[b].T.astype(ml_dtypes.bfloat16))
           for b in range(B)]
    in_maps = []
    for c in range(8):
        b, g = divmod(c, 4)
        qs, ks, vs = 256 * g, D + 256 * g, 2 * D + 256 * g
        wqk = np.ascontiguousarray(
            np.concatenate(
                [w_qkv[:, qs : qs + GD], w_qkv[:, ks : ks + GD]], axis=1
            ).astype(ml_dtypes.bfloat16)
        )
        wv = np.ascontiguousarray(
            w_qkv[:, vs : vs + GD].astype(ml_dtypes.bfloat16))
        wo = np.ascontiguousarray(
            w_out[256 * g : 256 * g + GD, :].astype(ml_dtypes.bfloat16))
        bqk = np.concatenate(
            [b_qkv[qs : qs + GD], b_qkv[ks : ks + GD]]
        ).reshape(2 * GD, 1).astype(np.float32)
        in_maps.append(
            {"xT": xTs[b], "wqk": wqk, "wv": wv, "wo": wo, "bqk": bqk,
             "maskT": maskT, "ident": ident, "vpat": make_vpat()}
        )
    return in_maps


_NC_CACHE = {}


def get_nc(L=2048):
    if L not in _NC_CACHE:
        _NC_CACHE[L] = build_nc(L)
    return _NC_CACHE[L]


def gather(results, b_qkv, w_out, b_out, L=2048):
    fix = (np.asarray(b_qkv, np.float32)[2 * D :] @ np.asarray(w_out, np.float32)
           + np.asarray(b_out, np.float32))
    y = np.zeros((B, L, D), np.float32)
    for c in range(8):
        b = c // 4
        y[b] += results[c]["y"]
    y += fix[None, None, :]
    return y


def kernel(x, w_qkv, b_qkv, w_out, b_out):
    L = x.shape[1]
    nc = get_nc(L)
    in_maps = shard_inputs(x, w_qkv, b_qkv, w_out, L=L)
    res = run_bass_kernel_spmd(nc, in_maps, core_ids=list(range(8)))
    return gather(res.results, b_qkv, w_out, b_out, L=L)


# revision 19
# speedup vs baseline: 1.0387x; 1.0387x over previous
"""Causal multi-head attention (B=2, L=2048, D=1024, H=16) on 8 trn2 cores.

Sharding: DP on batch (2) x TP on heads (4 groups of 4 heads) = 8 cores.
Each core computes, for its (batch b, head-group g):
  - qT/kT = wqk_g^T @ x_b^T            [512, L]   (head dims on partitions)
  - V     = x_b @ wv_g (+ ones cols)   [L, 4*65]  (natural layout, per-head ones
                                                   column so the PV matmul also
                                                   produces softmax denominators)
  - S^T   = K Q^T per (k-block, q-tile), causal-trimmed, both heads of a
            pair row-packed into one concurrent PE pass; additive causal mask
            (-240 pre-scale) accumulated into PSUM by a mask @ I matmul on
            the diagonal squares; ONE exp per k-block on ACT (3-D AP merging
            both heads' trimmed spans)
  - out^T = V_ext^T @ E^T accumulated over k-blocks  -> PSUM
            (partition 64 resp. 32 holds the softmax denominator r)
  - attn_raw^T evicted UNNORMALIZED (fast DVE copies only, so the PSUM
    accumulator ring is short); r rows staged to SBUF
  - normalization out-of-band: r broadcast across partitions via a
    step-0-free-dim SBUF->SBUF DMA, then attn = attn_raw / r elementwise
    on the otherwise-idle GpSimd engine (SBUF-only, which GpSimd requires)
  - y_part = attn @ w_out[rows of g]   [L, 1024]  (row-parallel out-proj)
Host gathers: y_b = sum_g y_part + (b_qkv_v @ w_out + b_out).

Emission is software-pipelined: pair-0 attention for all q-tiles runs right
after the m=0,2 n=0 projections it first needs; the remaining projection
tiles are emitted between attention windows sized so the PE stream stays
dense while ACT grinds exp; out-projection tiles pop into pair-1 windows.
"""

import sys
from contextlib import ExitStack

if "/opt/trn_rl_repo" not in sys.path:
    sys.path.insert(0, "/opt/trn_rl_repo")

import ml_dtypes
import numpy as np

import concourse.bass as bass
import concourse.mybir as mybir
import concourse.tile as tile
from concourse import bacc
from concourse.bass import ts
from concourse.bass_utils import run_bass_kernel_spmd

F32 = mybir.dt.float32
BF16 = mybir.dt.bfloat16
AF = mybir.ActivationFunctionType
OP = mybir.AluOpType

B, D, H = 2, 1024, 16
HD = 64           # head dim
NH = 4            # heads per core
GD = NH * HD      # 256 head dims per core
P = 128
QTW = 512         # q-tile width
VSTR = 193        # per-pair stride in the v tile: [V0(64)|1] + [z32|1|z31|V1(64)]
VW = 2 * VSTR     # v tile width (2 pairs)
MASKV = -240.0    # additive causal mask, pre-exp-scale (0.125) -> -30

NORM_GPSIMD = False  # divide on GpSimd (walrus rejects); Ln/Exp ACT + DVE mult


def bcast_ap(row_ap, n_part):
    """[1, N] SBUF AP -> (1, n_part, N) AP replicating the row (step-0 free
    dim), for DMA partition-broadcast."""
    from concourse.ap import AP

    dims = list(row_ap.ap)
    assert dims[0][1] == 1 and len(dims) == 2, dims
    return AP(row_ap.tensor, row_ap.offset,
              [list(dims[0]), [0, n_part], list(dims[1])])


def build_nc(L=2048):
    """Build the per-core Bass program. Same program for all 8 cores (SPMD)."""
    DK = D // P       # 8 contraction chunks
    LT = L // P       # l-tiles
    QT = L // QTW     # q-tiles
    QB = QTW // P     # k-blocks per q-tile (4)

    nc = bacc.Bacc("TRN2", target_bir_lowering=False, debug=False, num_devices=8)

    xT = nc.dram_tensor("xT", [D, L], BF16, kind="ExternalInput").ap()
    wqk = nc.dram_tensor("wqk", [D, 2 * GD], BF16, kind="ExternalInput").ap()
    wv = nc.dram_tensor("wv", [D, GD], BF16, kind="ExternalInput").ap()
    wo = nc.dram_tensor("wo", [GD, D], BF16, kind="ExternalInput").ap()
    bqk = nc.dram_tensor("bqk", [2 * GD, 1], F32, kind="ExternalInput").ap()
    maskT = nc.dram_tensor("maskT", [P, P], BF16, kind="ExternalInput").ap()
    ident = nc.dram_tensor("ident", [P, P], BF16, kind="ExternalInput").ap()
    # ones/zeros filler for the V slots: [1, 0*32, 1, 0*31] per partition
    vpat = nc.dram_tensor("vpat", [P, 65], BF16, kind="ExternalInput").ap()
    y = nc.dram_tensor("y", [L, D], F32, kind="ExternalOutput").ap()

    with tile.TileContext(nc) as tc, ExitStack() as stk:
        # ---------- persistent SBUF pools ----------
        const = stk.enter_context(tc.tile_pool(name="const", bufs=1))
        qk_pool = stk.enter_context(tc.tile_pool(name="qk", bufs=1))
        v_pool = stk.enter_context(tc.tile_pool(name="v", bufs=1))
        attn_pool = stk.enter_context(tc.tile_pool(name="attn", bufs=1))
        wo_pool = stk.enter_context(tc.tile_pool(name="wo", bufs=1))
        xt_pool = stk.enter_context(tc.tile_pool(name="xt", bufs=1))
        wi_pool = stk.enter_context(tc.tile_pool(name="wi", bufs=1))
        rs_pool = stk.enter_context(tc.tile_pool(name="rs", bufs=1))
        e_pool = stk.enter_context(tc.tile_pool(name="e", bufs=4))
        bc_pool = stk.enter_context(tc.tile_pool(name="bc", bufs=2))
        bc0_pool = stk.enter_context(tc.tile_pool(name="bc0", bufs=1))
        rv_pool = stk.enter_context(tc.tile_pool(name="rv", bufs=2))
        y_pool = stk.enter_context(tc.tile_pool(name="ysb", bufs=4))

        xt_sb = [xt_pool.tile([P, L], BF16, tag=f"xt{k}", name=f"xt_sb{k}") for k in range(DK)]
        wqk_sb = [wi_pool.tile([P, 2 * GD], BF16, tag=f"wqk{k}", name=f"wqk_sb{k}") for k in range(DK)]
        wv_sb = [wi_pool.tile([P, GD], BF16, tag=f"wv{k}", name=f"wv_sb{k}") for k in range(DK)]
        NT = L // QTW
        for k in range(DK):
            nc.sync.dma_start(wv_sb[k][:], wv[ts(k, P)])
        for k in range(DK):
            nc.sync.dma_start(wqk_sb[k][:], wqk[ts(k, P)])
            nc.sync.dma_start(xt_sb[k][:, ts(0, QTW)],
                              xT[ts(k, P), ts(0, QTW)])
        for n in range(1, NT):
            for k in range(DK):
                nc.sync.dma_start(xt_sb[k][:, ts(n, QTW)],
                                  xT[ts(k, P), ts(n, QTW)])

        bqk_sb = const.tile([P, 4], F32, tag="bqk", name="bqk_sb")
        for m in range(4):
            nc.sync.dma_start(bqk_sb[:, m : m + 1], bqk[ts(m, P)])
        maskT_sb = const.tile([P, P], BF16, tag="maskT", name="maskT_sb")
        nc.sync.dma_start(maskT_sb[:], maskT)
        ident_sb = const.tile([P, P], BF16, tag="ident", name="ident_sb")
        nc.sync.dma_start(ident_sb[:], ident)

        # m-tile 0,1 = qT (head pairs 01, 23); 2,3 = kT
        qk_sb = [qk_pool.tile([P, L], BF16, tag=f"qk{m}", name=f"qk_sb{m}") for m in range(4)]
        v_sb = [v_pool.tile([P, VW], BF16, tag=f"v{t}", name=f"v_sb{t}") for t in range(LT)]
        attn_sb = [attn_pool.tile([P, L], BF16, tag=f"attn{p}", name=f"attn_sb{p}") for p in range(2)]
        rs_sb = [rs_pool.tile([P, L], BF16, tag=f"rs{p}", name=f"rs_sb{p}") for p in range(2)]
        wo_sb = [wo_pool.tile([P, D], BF16, tag=f"wo{c}", name=f"wo_sb{c}") for c in range(2)]
        for c in range(2):
            nc.sync.dma_start(wo_sb[c][:], wo[ts(c, P)])
        for lt in range(LT):
            vv = v_sb[lt][:, 0:VW].rearrange("p (a c) -> p a c", a=2, c=VSTR)
            for a in range(2):
                nc.sync.dma_start(vv[:, a, 64:129], vpat)

        # ---------- PSUM pools (LIFO lifetimes) ----------
        # pss (4 banks): attention scores; also hosts the V accumulators in
        # phase A (4x 256-wide regions per [P, 1024] tile).
        # pso (2 banks): PV accumulators.  psq (2 banks): q/k projection
        # tiles; closed after the pair-0 sweep and replaced by psy (out-proj).
        pss_cm = tc.tile_pool(name="pss", bufs=2, space="PSUM")
        pss = pss_cm.__enter__()
        pso_cm = tc.tile_pool(name="pso", bufs=1, space="PSUM")
        pso = pso_cm.__enter__()
        psq_cm = tc.tile_pool(name="psq", bufs=1, space="PSUM")
        psq = psq_cm.__enter__()

        # ---------- V projection: per n-block of 4 l-tiles ----------
        # Accumulators live in the pss slots, one group per PSUM bank
        # (regions at 512-f32 bank starts), k-outer to ride the x DMA.
        def v_block(n):
            pvt = [pss.tile([P, 2 * QTW], F32, tag="pss", name=f"ps_v{i}")
                   for i in range(2)]
            regs = [pvt[i // 2][:, QTW * (i % 2) : QTW * (i % 2) + GD]
                    for i in range(4)]
            lts = list(range(4 * n, 4 * n + 4))
            for k in range(DK):
                for i, lt in enumerate(lts):
                    nc.tensor.matmul(
                        regs[i],
                        xt_sb[k][:, ts(lt, P)],
                        wv_sb[k][:],
                        start=(k == 0),
                        stop=(k == DK - 1),
                    )
            for i, lt in enumerate(lts):
                vv = v_sb[lt][:, 0:VW].rearrange("p (a c) -> p a c", a=2, c=VSTR)
                pv = regs[i].rearrange("p (a c) -> p a c", a=2, c=2 * HD)
                if lt % 2 == 0:
                    nc.vector.tensor_copy(vv[:, :, 0:64], pv[:, :, 0:64])
                    nc.vector.tensor_copy(vv[:, :, 129:193], pv[:, :, 64:128])
                else:
                    nc.scalar.copy(vv[:, :, 0:64], pv[:, :, 0:64])
                    nc.scalar.copy(vv[:, :, 129:193], pv[:, :, 64:128])

        # ---------- q/k projection tiles (each: one [P,QTW] n-slice of m) ----
        # qT/kT: [512, L] = wqk^T @ xT, bias added during PSUM eviction.
        qk_done = set()

        def qk_tile(m, n):
            ps = psq.tile([P, QTW], F32, tag=f"psq{(m + n) % 2}", name="ps_q")
            for k in range(DK):
                nc.tensor.matmul(
                    ps[:],
                    wqk_sb[k][:, ts(m, P)],
                    xt_sb[k][:, ts(n, QTW)],
                    start=(k == 0),
                    stop=(k == DK - 1),
                )
            nc.vector.tensor_scalar(
                out=qk_sb[m][:, ts(n, QTW)],
                in0=ps[:],
                scalar1=bqk_sb[:, m : m + 1],
                scalar2=None,
                op0=OP.add,
            )
            qk_done.add((m, n))

        SKEW = 3

        def attn_raw(qt, pair, fillers=()):
            """Attention for (q-tile qt, head-pair pair): scores+mask+exp+PV,
            then a fast UNNORMALIZED eviction (attn_raw + r rows). `fillers`
            is a list of (unit_idx, closure) to emit between units."""
            q_t = qk_sb[pair]
            k_t = qk_sb[2 + pair]
            assert (pair, qt) in qk_done and (2 + pair, qt) in qk_done
            out_ps = [
                pso.tile([P, QTW], F32, tag=f"pso{h}", name=f"ps_o{h}")
                for h in range(2)
            ]
            nblk = QB * qt + QB     # k-blocks for this q-tile
            fill = {}
            for u, clo in fillers:
                fill.setdefault(u % nblk, []).append(clo)

            def front(j):
                sp = pss.tile([P, 2 * QTW], F32, tag="pss", name="ps_s")
                e_t = e_pool.tile([P, 2 * QTW], BF16, tag="e", name="e_t")
                diag = j >= QB * qt
                da = (j - QB * qt) * P if diag else 0
                for hl in range(2):
                    hb = 64 * hl
                    nc.tensor.matmul(
                        sp[:, hl * QTW + da : (hl + 1) * QTW],
                        k_t[hb : hb + 64, ts(j, P)],
                        q_t[hb : hb + 64,
                            qt * QTW + da : (qt + 1) * QTW],
                        start=True, stop=True)
                if diag:
                    # additive causal mask on the diagonal square:
                    # sp[:, da:da+P] += maskT.T @ I  (strict upper = -240)
                    for hl in range(2):
                        nc.tensor.matmul(
                            sp[:, hl * QTW + da : hl * QTW + da + P],
                            maskT_sb[:],
                            ident_sb[:],
                            start=False, stop=True,
                            skip_group_check=True)
                if da == 0:
                    nc.scalar.activation(e_t[:], sp[:], AF.Exp, scale=0.125)
                else:
                    spv = sp[:].rearrange("p (h c) -> p h c", h=2, c=QTW)
                    etv = e_t[:].rearrange("p (h c) -> p h c", h=2, c=QTW)
                    nc.scalar.activation(etv[:, :, da:QTW], spv[:, :, da:QTW],
                                         AF.Exp, scale=0.125)

                def emit_pv(j=j, da=da, e_t=e_t):
                    for hl in range(2):
                        mm_pv(nc, out_ps[hl], hl, da,
                              vext(v_sb[j], pair, hl),
                              e_t[:, hl * QTW + da : (hl + 1) * QTW],
                              start=(j == 0), stop=(j == nblk - 1))
                return emit_pv

            pend = []
            for j in range(nblk):
                pend.append(front(j))
                for clo in fill.get(j, ()):
                    clo()
                if j >= SKEW:
                    pend[j - SKEW]()
            for j in range(max(0, nblk - SKEW), nblk):
                pend[j]()

            # fast raw eviction: attn_raw (bf16) + r rows -> rs (bf16). The
            # out_ps ring contains only these four DVE copies.
            nc.vector.tensor_copy(attn_sb[pair][0:64, ts(qt, QTW)],
                                  out_ps[0][0:64, :])
            nc.vector.tensor_copy(attn_sb[pair][64:P, ts(qt, QTW)],
                                  out_ps[1][64:P, :])
            nc.vector.tensor_copy(rs_sb[pair][64:65, ts(qt, QTW)],
                                  out_ps[0][64:65, :])
            nc.vector.tensor_copy(rs_sb[pair][32:33, ts(qt, QTW)],
                                  out_ps[1][32:33, :])

        def norm(pair, q0, nqt):
            """Normalize attn_sb[pair][:, q0*QTW : (q0+nqt)*QTW] by its r."""
            w = nqt * QTW
            sl = slice(q0 * QTW, q0 * QTW + w)
            if NORM_GPSIMD:
                if nqt > 1:
                    bc = bc0_pool.tile([P, w], BF16, tag="bc0", name="bc0_t")
                else:
                    bc = bc_pool.tile([P, QTW], BF16, tag="bc", name="bc_t")
                nc.sync.dma_start(bc[0:64, 0:w],
                                  bcast_ap(rs_sb[pair][64:65, sl], 64))
                nc.sync.dma_start(bc[64:P, 0:w],
                                  bcast_ap(rs_sb[pair][32:33, sl], 64))
                nc.gpsimd.tensor_tensor(
                    out=attn_sb[pair][:, sl],
                    in0=attn_sb[pair][:, sl],
                    in1=bc[:, 0:w], op=OP.divide)
            else:
                rv = rv_pool.tile([P, w], F32, tag=f"rv{w}", name="rv_t")
                if nqt > 1:
                    bc = bc0_pool.tile([P, w], F32, tag="bc0", name="bc0_t")
                else:
                    bc = bc_pool.tile([P, QTW], F32, tag="bc", name="bc_t")
                nc.scalar.activation(rv[64:65, 0:w], rs_sb[pair][64:65, sl],
                                     AF.Ln)
                nc.scalar.activation(rv[32:33, 0:w], rs_sb[pair][32:33, sl],
                                     AF.Ln)
                nc.scalar.activation(rv[:, 0:w], rv[:, 0:w], AF.Exp,
                                     scale=-1.0)
                nc.sync.dma_start(bc[0:64, 0:w], bcast_ap(rv[64:65, 0:w], 64))
                nc.sync.dma_start(bc[64:P, 0:w], bcast_ap(rv[32:33, 0:w], 64))
                nc.vector.tensor_tensor(
                    out=attn_sb[pair][0:64, sl],
                    in0=attn_sb[pair][0:64, sl], in1=bc[0:64, 0:w],
                    op=OP.mult)
                nc.vector.tensor_tensor(
                    out=attn_sb[pair][64:P, sl],
                    in0=attn_sb[pair][64:P, sl], in1=bc[64:P, 0:w],
                    op=OP.mult)

        # ---------- pair-0 sweep (V + projection tiles interleaved) ----------
        v_block(0)
        qk_tile(0, 0)
        qk_tile(2, 0)
        attn_raw(0, 0)
        v_block(1)
        qk_tile(0, 1)
        qk_tile(2, 1)
        attn_raw(1, 0, fillers=[
            (2, lambda: qk_tile(1, 0)), (5, lambda: qk_tile(3, 0))])
        v_block(2)
        qk_tile(0, 2)
        qk_tile(2, 2)
        attn_raw(2, 0, fillers=[
            (2, lambda: qk_tile(1, 1)), (6, lambda: qk_tile(3, 1))])
        v_block(3)
        qk_tile(0, 3)
        qk_tile(2, 3)
        attn_raw(3, 0, fillers=[
            (2, lambda: qk_tile(1, 2)), (5, lambda: qk_tile(3, 2)),
            (8, lambda: qk_tile(1, 3)), (11, lambda: qk_tile(3, 3))])
        norm(0, 0, QT)

        # q/k psum space becomes the out-projection pool
        psq_cm.__exit__(None, None, None)
        psy_cm = tc.tile_pool(name="psy", bufs=2, space="PSUM")
        psy = psy_cm.__enter__()

        # ---------- pair-1 sweep (out-projection tiles interleaved) ------
        nyd = [0]

        def op_tile(lt, nh):
            ps = psy.tile([P, QTW], F32, tag="psy", name="ps_y")
            for c in range(2):
                nc.tensor.matmul(
                    ps[:],
                    attn_sb[c][:, ts(lt, P)],
                    wo_sb[c][:, ts(nh, QTW)],
                    start=(c == 0),
                    stop=(c == 1),
                )
            yt = y_pool.tile([P, QTW], F32, tag="y", name="y_t")
            nyd[0] += 1
            if nyd[0] > 24:        # tail tiles: ACT is idle by then
                nc.scalar.copy(yt[:], ps[:])
            else:
                nc.vector.tensor_copy(yt[:], ps[:])
            nc.sync.dma_start(y[ts(lt, P), ts(nh, QTW)], yt[:])

        def op_fillers(qt, units):
            out = []
            tiles = [(lt, nh) for lt in range(4 * qt, 4 * qt + 4)
                     for nh in range(2)]
            step = max(1, units // len(tiles))
            for i, (lt, nh) in enumerate(tiles):
                out.append((1 + i * step,
                            lambda lt=lt, nh=nh: op_tile(lt, nh)))
            return out

        attn_raw(0, 1)
        norm(1, 0, 1)
        attn_raw(1, 1, fillers=op_fillers(0, 8))
        norm(1, 1, 1)
        attn_raw(2, 1, fillers=op_fillers(1, 12))
        norm(1, 2, 1)
        attn_raw(3, 1, fillers=op_fillers(2, 16))
        norm(1, 3, 1)
        for lt in range(12, 16):
            for nh in range(2):
                op_tile(lt, nh)

        psy_cm.__exit__(None, None, None)
        pso_cm.__exit__(None, None, None)
        pss_cm.__exit__(None, None, None)

    nc.compile()
    return nc


def vext(vt, pair, hl):
    """lhsT slice of the extended-V tile for (pair, local head hl)."""
    base = VSTR * pair
    if hl == 0:
        return vt[:, base : base + 65]          # M=65: V at 0-63, r at 64
    return vt[:, base + 65 : base + VSTR]       # M=128: ones@32, V at 64-127


def mm_pv(nc, out_ps, hl, c0, lhsT, rhs, start, stop):
    if hl == 0:
        out = out_ps[0:65, c0:QTW]
    else:
        out = out_ps[:, c0:QTW]
    nc.tensor.matmul(out, lhsT, rhs, start=start, stop=stop)


def make_maskT():
    # additive mask: want  (maskT.T @ I)[k, c] = MASKV if k > c else 0
    # => maskT[c, k] = MASKV for k > c: strict upper triangle.
    m = np.zeros((P, P), np.float32)
    m[np.arange(P)[:, None] < np.arange(P)[None, :]] = MASKV
    return m.astype(ml_dtypes.bfloat16)


def make_vpat():
    pat = np.zeros((P, 65), ml_dtypes.bfloat16)
    pat[:, 0] = 1.0   # even-head ones col (tile col 64): r -> partition 64
    pat[:, 33] = 1.0  # odd-head ones col (tile col 97): r -> partition 32
    return pat


def shard_inputs(x, w_qkv, b_qkv, w_out, L=2048):
    """Host-side sharding: core c = (batch c//4, head-group c%4)."""
    x = np.asarray(x, np.float32)
    w_qkv = np.asarray(w_qkv, np.float32)
    b_qkv = np.asarray(b_qkv, np.float32)
    w_out = np.asarray(w_out, np.float32)
    ident = np.eye(P, dtype=ml_dtypes.bfloat16)
    maskT = make_maskT()
    xTs = [np.ascontiguousarray(x[b].T.astype(ml_dtypes.bfloat16))
           for b in range(B)]
    in_maps = []
    for c in range(8):
        b, g = divmod(c, 4)
        qs, ks, vs = 256 * g, D + 256 * g, 2 * D + 256 * g
        wqk = np.ascontiguousarray(
            np.concatenate(
                [w_qkv[:, qs : qs + GD], w_qkv[:, ks : ks + GD]], axis=1
            ).astype(ml_dtypes.bfloat16)
        )
        wv = np.ascontiguousarray(
            w_qkv[:, vs : vs + GD].astype(ml_dtypes.bfloat16))
        wo = np.ascontiguousarray(
            w_out[256 * g : 256 * g + GD, :].astype(ml_dtypes.bfloat16))
        bqk = np.concatenate(
            [b_qkv[qs : qs + GD], b_qkv[ks : ks + GD]]
        ).reshape(2 * GD, 1).astype(np.float32)
        in_maps.append(
            {"xT": xTs[b], "wqk": wqk, "wv": wv, "wo": wo, "bqk": bqk,
             "maskT": maskT, "ident": ident, "vpat": make_vpat()}
        )
    return in_maps


_NC_CACHE = {}


def get_nc(L=2048):
    if L not in _NC_CACHE:
        _NC_CACHE[L] = build_nc(L)
    return _NC_CACHE[L]


def gather(results, b_qkv, w_out, b_out, L=2048):
    fix = (np.asarray(b_qkv, np.float32)[2 * D :] @ np.asarray(w_out, np.float32)
           + np.asarray(b_out, np.float32))
    y = np.zeros((B, L, D), np.float32)
    for c in range(8):
        b = c // 4
        y[b] += results[c]["y"]
    y += fix[None, None, :]
    return y


def kernel(x, w_qkv, b_qkv, w_out, b_out):
    L = x.shape[1]
    nc = get_nc(L)
    in_maps = shard_inputs(x, w_qkv, b_qkv, w_out, L=L)
    res = run_bass_kernel_spmd(nc, in_maps, core_ids=list(range(8)))
    return gather(res.results, b_qkv, w_out, b_out, L=L)


# revision 23
# speedup vs baseline: 1.0802x; 1.0399x over previous
"""Causal multi-head attention (B=2, L=2048, D=1024, H=16) on 8 trn2 cores.

Sharding: DP on batch (2) x TP on heads (4 groups of 4 heads) = 8 cores.
Each core computes, for its (batch b, head-group g):
  - qT/kT = wqk_g^T @ x_b^T            [512, L]   (head dims on partitions)
  - V     = x_b @ wv_g (+ ones cols)   [L, 4*65]  (natural layout, per-head ones
                                                   column so the PV matmul also
                                                   produces softmax denominators)
  - S^T   = K Q^T per (k-block, q-tile), causal-trimmed, both heads of a
            pair row-packed into one concurrent PE pass; additive causal mask
            (-240 pre-scale) accumulated into PSUM by a mask @ I matmul on
            the diagonal squares; ONE exp per k-block on ACT (3-D AP merging
            both heads' trimmed spans)
  - out^T = V_ext^T @ E^T accumulated over k-blocks  -> PSUM
            (partition 64 resp. 32 holds the softmax denominator r)
  - attn_raw^T evicted UNNORMALIZED (fast DVE copies only, so the PSUM
    accumulator ring is short); r rows staged to SBUF
  - normalization out-of-band: r broadcast across partitions via a
    step-0-free-dim SBUF->SBUF DMA, then attn = attn_raw / r elementwise
    on the otherwise-idle GpSimd engine (SBUF-only, which GpSimd requires)
  - y_part = attn @ w_out[rows of g]   [L, 1024]  (row-parallel out-proj)
Host gathers: y_b = sum_g y_part + (b_qkv_v @ w_out + b_out).

Emission is software-pipelined: pair-0 attention for all q-tiles runs right
after the m=0,2 n=0 projections it first needs; the remaining projection
tiles are emitted between attention windows sized so the PE stream stays
dense while ACT grinds exp; out-projection tiles pop into pair-1 windows.
"""

import sys
from contextlib import ExitStack

if "/opt/trn_rl_repo" not in sys.path:
    sys.path.insert(0, "/opt/trn_rl_repo")

import ml_dtypes
import numpy as np

import concourse.bass as bass
import concourse.mybir as mybir
import concourse.tile as tile
from concourse import bacc
from concourse.bass import ts
from concourse.bass_utils import run_bass_kernel_spmd

F32 = mybir.dt.float32
BF16 = mybir.dt.bfloat16
AF = mybir.ActivationFunctionType
OP = mybir.AluOpType

B, D, H = 2, 1024, 16
HD = 64           # head dim
NH = 4            # heads per core
GD = NH * HD      # 256 head dims per core
P = 128
QTW = 512         # q-tile width
VSTR = 193        # per-pair stride in the v tile: [V0(64)|1] + [z32|1|z31|V1(64)]
VW = 2 * VSTR     # v tile width (2 pairs)
MASKV = -240.0    # additive causal mask, pre-exp-scale (0.125) -> -30

NORM_GPSIMD = False  # divide on GpSimd (walrus rejects); Ln/Exp ACT + DVE mult


def bcast_ap(row_ap, n_part):
    """[1, N] SBUF AP -> (1, n_part, N) AP replicating the row (step-0 free
    dim), for DMA partition-broadcast."""
    from concourse.ap import AP

    dims = list(row_ap.ap)
    assert dims[0][1] == 1 and len(dims) == 2, dims
    return AP(row_ap.tensor, row_ap.offset,
              [list(dims[0]), [0, n_part], list(dims[1])])


def build_nc(L=2048):
    """Build the per-core Bass program. Same program for all 8 cores (SPMD)."""
    DK = D // P       # 8 contraction chunks
    LT = L // P       # l-tiles
    QT = L // QTW     # q-tiles
    QB = QTW // P     # k-blocks per q-tile (4)

    nc = bacc.Bacc("TRN2", target_bir_lowering=False, debug=False, num_devices=8)

    xT = nc.dram_tensor("xT", [D, L], BF16, kind="ExternalInput").ap()
    wqk = nc.dram_tensor("wqk", [D, 2 * GD], BF16, kind="ExternalInput").ap()
    wv = nc.dram_tensor("wv", [D, GD], BF16, kind="ExternalInput").ap()
    wo = nc.dram_tensor("wo", [GD, D], BF16, kind="ExternalInput").ap()
    bqk = nc.dram_tensor("bqk", [2 * GD, 1], F32, kind="ExternalInput").ap()
    maskT = nc.dram_tensor("maskT", [P, P], BF16, kind="ExternalInput").ap()
    ident = nc.dram_tensor("ident", [P, P], BF16, kind="ExternalInput").ap()
    # ones/zeros filler for the V slots: [1, 0*32, 1, 0*31] per partition
    vpat = nc.dram_tensor("vpat", [P, 65], BF16, kind="ExternalInput").ap()
    y = nc.dram_tensor("y", [L, D], F32, kind="ExternalOutput").ap()

    with tile.TileContext(nc) as tc, ExitStack() as stk:
        # ---------- persistent SBUF pools ----------
        const = stk.enter_context(tc.tile_pool(name="const", bufs=1))
        qk_pool = stk.enter_context(tc.tile_pool(name="qk", bufs=1))
        v_pool = stk.enter_context(tc.tile_pool(name="v", bufs=1))
        attn_pool = stk.enter_context(tc.tile_pool(name="attn", bufs=1))
        wo_pool = stk.enter_context(tc.tile_pool(name="wo", bufs=1))
        xt_pool = stk.enter_context(tc.tile_pool(name="xt", bufs=1))
        wi_pool = stk.enter_context(tc.tile_pool(name="wi", bufs=1))
        rs_pool = stk.enter_context(tc.tile_pool(name="rs", bufs=1))
        e_pool = stk.enter_context(tc.tile_pool(name="e", bufs=4))
        bc_pool = stk.enter_context(tc.tile_pool(name="bc", bufs=2))
        bc0_pool = stk.enter_context(tc.tile_pool(name="bc0", bufs=1))
        rv_pool = stk.enter_context(tc.tile_pool(name="rv", bufs=2))
        y_pool = stk.enter_context(tc.tile_pool(name="ysb", bufs=4))

        xt_sb = [xt_pool.tile([P, L], BF16, tag=f"xt{k}", name=f"xt_sb{k}") for k in range(DK)]
        wqk_sb = [wi_pool.tile([P, 2 * GD], BF16, tag=f"wqk{k}", name=f"wqk_sb{k}") for k in range(DK)]
        wv_sb = [wi_pool.tile([P, GD], BF16, tag=f"wv{k}", name=f"wv_sb{k}") for k in range(DK)]
        NT = L // QTW
        for k in range(DK):
            nc.sync.dma_start(wv_sb[k][:], wv[ts(k, P)])
        for k in range(DK):
            nc.sync.dma_start(wqk_sb[k][:], wqk[ts(k, P)])
            nc.sync.dma_start(xt_sb[k][:, ts(0, QTW)],
                              xT[ts(k, P), ts(0, QTW)])
        for n in range(1, NT):
            for k in range(DK):
                nc.sync.dma_start(xt_sb[k][:, ts(n, QTW)],
                                  xT[ts(k, P), ts(n, QTW)])

        bqk_sb = const.tile([P, 4], F32, tag="bqk", name="bqk_sb")
        for m in range(4):
            nc.sync.dma_start(bqk_sb[:, m : m + 1], bqk[ts(m, P)])
        maskT_sb = const.tile([P, P], BF16, tag="maskT", name="maskT_sb")
        nc.sync.dma_start(maskT_sb[:], maskT)
        ident_sb = const.tile([P, P], BF16, tag="ident", name="ident_sb")
        nc.sync.dma_start(ident_sb[:], ident)

        # m-tile 0,1 = qT (head pairs 01, 23); 2,3 = kT
        qk_sb = [qk_pool.tile([P, L], BF16, tag=f"qk{m}", name=f"qk_sb{m}") for m in range(4)]
        v_sb = [v_pool.tile([P, VW], BF16, tag=f"v{t}", name=f"v_sb{t}") for t in range(LT)]
        attn_sb = [attn_pool.tile([P, L], BF16, tag=f"attn{p}", name=f"attn_sb{p}") for p in range(2)]
        rs_sb = [rs_pool.tile([P, L], BF16, tag=f"rs{p}", name=f"rs_sb{p}") for p in range(2)]
        wo_sb = [wo_pool.tile([P, D], BF16, tag=f"wo{c}", name=f"wo_sb{c}") for c in range(2)]
        for c in range(2):
            nc.sync.dma_start(wo_sb[c][:], wo[ts(c, P)])
        for lt in range(LT):
            vv = v_sb[lt][:, 0:VW].rearrange("p (a c) -> p a c", a=2, c=VSTR)
            for a in range(2):
                nc.sync.dma_start(vv[:, a, 64:129], vpat)

        # ---------- PSUM pools (LIFO lifetimes) ----------
        # pss (4 banks): attention scores; also hosts the V accumulators in
        # phase A (4x 256-wide regions per [P, 1024] tile).
        # pso (2 banks): PV accumulators.  psq (2 banks): q/k projection
        # tiles; closed after the pair-0 sweep and replaced by psy (out-proj).
        pss_cm = tc.tile_pool(name="pss", bufs=2, space="PSUM")
        pss = pss_cm.__enter__()
        pso_cm = tc.tile_pool(name="pso", bufs=1, space="PSUM")
        pso = pso_cm.__enter__()
        psq_cm = tc.tile_pool(name="psq", bufs=1, space="PSUM")
        psq = psq_cm.__enter__()

        # ---------- V projection: per n-block of 4 l-tiles ----------
        # Accumulators live in the pss slots, one group per PSUM bank
        # (regions at 512-f32 bank starts), k-outer to ride the x DMA.
        def v_block(n):
            pvt = [pss.tile([P, 2 * QTW], F32, tag="pss", name=f"ps_v{i}")
                   for i in range(2)]
            regs = [pvt[i // 2][:, QTW * (i % 2) : QTW * (i % 2) + GD]
                    for i in range(4)]
            lts = list(range(4 * n, 4 * n + 4))
            for k in range(DK):
                for i, lt in enumerate(lts):
                    nc.tensor.matmul(
                        regs[i],
                        xt_sb[k][:, ts(lt, P)],
                        wv_sb[k][:],
                        start=(k == 0),
                        stop=(k == DK - 1),
                    )
            for i, lt in enumerate(lts):
                vv = v_sb[lt][:, 0:VW].rearrange("p (a c) -> p a c", a=2, c=VSTR)
                pv = regs[i].rearrange("p (a c) -> p a c", a=2, c=2 * HD)
                if lt % 2 == 0:
                    nc.vector.tensor_copy(vv[:, :, 0:64], pv[:, :, 0:64])
                    nc.vector.tensor_copy(vv[:, :, 129:193], pv[:, :, 64:128])
                else:
                    nc.scalar.copy(vv[:, :, 0:64], pv[:, :, 0:64])
                    nc.scalar.copy(vv[:, :, 129:193], pv[:, :, 64:128])

        # ---------- q/k projection tiles (each: one [P,QTW] n-slice of m) ----
        # qT/kT: [512, L] = wqk^T @ xT, bias added during PSUM eviction.
        qk_done = set()

        def qk_tile(m, n):
            ps = psq.tile([P, QTW], F32, tag=f"psq{(m + n) % 2}", name="ps_q")
            for k in range(DK):
                nc.tensor.matmul(
                    ps[:],
                    wqk_sb[k][:, ts(m, P)],
                    xt_sb[k][:, ts(n, QTW)],
                    start=(k == 0),
                    stop=(k == DK - 1),
                )
            nc.vector.tensor_scalar(
                out=qk_sb[m][:, ts(n, QTW)],
                in0=ps[:],
                scalar1=bqk_sb[:, m : m + 1],
                scalar2=None,
                op0=OP.add,
            )
            qk_done.add((m, n))

        SKEW = 3

        def attn_raw(qt, pair, fillers=()):
            """Attention for (q-tile qt, head-pair pair): scores+mask+exp+PV,
            then a fast UNNORMALIZED eviction (attn_raw + r rows). `fillers`
            is a list of (unit_idx, closure) to emit between units."""
            q_t = qk_sb[pair]
            k_t = qk_sb[2 + pair]
            assert (pair, qt) in qk_done and (2 + pair, qt) in qk_done
            out_ps = [
                pso.tile([P, QTW], F32, tag=f"pso{h}", name=f"ps_o{h}")
                for h in range(2)
            ]
            nblk = QB * qt + QB     # k-blocks for this q-tile
            fill = {}
            for u, clo in fillers:
                fill.setdefault(u % nblk, []).append(clo)

            def front(j):
                sp = pss.tile([P, 2 * QTW], F32, tag="pss", name="ps_s")
                e_t = e_pool.tile([P, 2 * QTW], BF16, tag="e", name="e_t")
                diag = j >= QB * qt
                da = (j - QB * qt) * P if diag else 0
                for hl in range(2):
                    hb = 64 * hl
                    nc.tensor.matmul(
                        sp[:, hl * QTW + da : (hl + 1) * QTW],
                        k_t[hb : hb + 64, ts(j, P)],
                        q_t[hb : hb + 64,
                            qt * QTW + da : (qt + 1) * QTW],
                        start=True, stop=True)
                if diag:
                    # additive causal mask on the diagonal square:
                    # sp[:, da:da+P] += maskT.T @ I  (strict upper = -240)
                    for hl in range(2):
                        nc.tensor.matmul(
                            sp[:, hl * QTW + da : hl * QTW + da + P],
                            maskT_sb[:],
                            ident_sb[:],
                            start=False, stop=True,
                            skip_group_check=True)
                if da == 0:
                    nc.scalar.activation(e_t[:], sp[:], AF.Exp, scale=0.125)
                else:
                    spv = sp[:].rearrange("p (h c) -> p h c", h=2, c=QTW)
                    etv = e_t[:].rearrange("p (h c) -> p h c", h=2, c=QTW)
                    nc.scalar.activation(etv[:, :, da:QTW], spv[:, :, da:QTW],
                                         AF.Exp, scale=0.125)

                def emit_pv(j=j, da=da, e_t=e_t):
                    for hl in range(2):
                        mm_pv(nc, out_ps[hl], hl, da,
                              vext(v_sb[j], pair, hl),
                              e_t[:, hl * QTW + da : (hl + 1) * QTW],
                              start=(j == 0), stop=(j == nblk - 1))
                return emit_pv

            pend = []
            for j in range(nblk):
                pend.append(front(j))
                for clo in fill.get(j, ()):
                    clo()
                if j >= SKEW:
                    pend[j - SKEW]()
            for j in range(max(0, nblk - SKEW), nblk):
                pend[j]()

            # fast raw eviction: attn_raw (bf16) + r rows -> rs (bf16). The
            # out_ps ring contains only these four DVE copies.
            nc.vector.tensor_copy(attn_sb[pair][0:64, ts(qt, QTW)],
                                  out_ps[0][0:64, :])
            nc.vector.tensor_copy(attn_sb[pair][64:P, ts(qt, QTW)],
                                  out_ps[1][64:P, :])
            nc.vector.tensor_copy(rs_sb[pair][64:65, ts(qt, QTW)],
                                  out_ps[0][64:65, :])
            nc.vector.tensor_copy(rs_sb[pair][32:33, ts(qt, QTW)],
                                  out_ps[1][32:33, :])

        def norm(pair, q0, nqt):
            """Normalize attn_sb[pair][:, q0*QTW : (q0+nqt)*QTW] by its r."""
            w = nqt * QTW
            sl = slice(q0 * QTW, q0 * QTW + w)
            if NORM_GPSIMD:
                if nqt > 1:
                    bc = bc0_pool.tile([P, w], BF16, tag="bc0", name="bc0_t")
                else:
                    bc = bc_pool.tile([P, QTW], BF16, tag="bc", name="bc_t")
                nc.sync.dma_start(bc[0:64, 0:w],
                                  bcast_ap(rs_sb[pair][64:65, sl], 64))
                nc.sync.dma_start(bc[64:P, 0:w],
                                  bcast_ap(rs_sb[pair][32:33, sl], 64))
                nc.gpsimd.tensor_tensor(
                    out=attn_sb[pair][:, sl],
                    in0=attn_sb[pair][:, sl],
                    in1=bc[:, 0:w], op=OP.divide)
            else:
                rv = rv_pool.tile([P, w], F32, tag=f"rv{w}", name="rv_t")
                if nqt > 1:
                    bc = bc0_pool.tile([P, w], F32, tag="bc0", name="bc0_t")
                else:
                    bc = bc_pool.tile([P, QTW], F32, tag="bc", name="bc_t")
                nc.scalar.activation(rv[64:65, 0:w], rs_sb[pair][64:65, sl],
                                     AF.Ln)
                nc.scalar.activation(rv[32:33, 0:w], rs_sb[pair][32:33, sl],
                                     AF.Ln)
                nc.scalar.activation(rv[64:65, 0:w], rv[64:65, 0:w], AF.Exp,
                                     scale=-1.0)
                nc.scalar.activation(rv[32:33, 0:w], rv[32:33, 0:w], AF.Exp,
                                     scale=-1.0)
                nc.sync.dma_start(bc[0:64, 0:w], bcast_ap(rv[64:65, 0:w], 64))
                nc.sync.dma_start(bc[64:P, 0:w], bcast_ap(rv[32:33, 0:w], 64))
                nc.vector.tensor_tensor(
                    out=attn_sb[pair][0:64, sl],
                    in0=attn_sb[pair][0:64, sl], in1=bc[0:64, 0:w],
                    op=OP.mult)
                nc.vector.tensor_tensor(
                    out=attn_sb[pair][64:P, sl],
                    in0=attn_sb[pair][64:P, sl], in1=bc[64:P, 0:w],
                    op=OP.mult)

        # ---------- pair-0 sweep (V + projection tiles interleaved) ----------
        v_block(0)
        qk_tile(0, 0)
        qk_tile(2, 0)
        attn_raw(0, 0)
        v_block(1)
        qk_tile(0, 1)
        qk_tile(2, 1)
        attn_raw(1, 0, fillers=[
            (2, lambda: qk_tile(1, 0)), (5, lambda: qk_tile(3, 0))])
        v_block(2)
        qk_tile(0, 2)
        qk_tile(2, 2)
        attn_raw(2, 0, fillers=[
            (2, lambda: qk_tile(1, 1)), (6, lambda: qk_tile(3, 1))])
        v_block(3)
        qk_tile(0, 3)
        qk_tile(2, 3)
        attn_raw(3, 0, fillers=[
            (2, lambda: qk_tile(1, 2)), (5, lambda: qk_tile(3, 2)),
            (8, lambda: qk_tile(1, 3)), (11, lambda: qk_tile(3, 3))])
        for qt in range(QT):
            norm(0, qt, 1)

        # q/k psum space becomes the out-projection pool
        psq_cm.__exit__(None, None, None)
        psy_cm = tc.tile_pool(name="psy", bufs=2, space="PSUM")
        psy = psy_cm.__enter__()

        # ---------- pair-1 sweep (out-projection tiles interleaved) ------
        nyd = [0]

        def op_tile(lt, nh):
            ps = psy.tile([P, QTW], F32, tag="psy", name="ps_y")
            for c in range(2):
                nc.tensor.matmul(
                    ps[:],
                    attn_sb[c][:, ts(lt, P)],
                    wo_sb[c][:, ts(nh, QTW)],
                    start=(c == 0),
                    stop=(c == 1),
                )
            yt = y_pool.tile([P, QTW], F32, tag="y", name="y_t")
            nyd[0] += 1
            if nyd[0] > 24:        # tail tiles: ACT is idle by then
                nc.scalar.copy(yt[:], ps[:])
            else:
                nc.vector.tensor_copy(yt[:], ps[:])
            nc.sync.dma_start(y[ts(lt, P), ts(nh, QTW)], yt[:])

        def op_fillers(qt, units):
            out = []
            tiles = [(lt, nh) for lt in range(4 * qt, 4 * qt + 4)
                     for nh in range(2)]
            step = max(1, units // len(tiles))
            for i, (lt, nh) in enumerate(tiles):
                out.append((1 + i * step,
                            lambda lt=lt, nh=nh: op_tile(lt, nh)))
            return out

        attn_raw(0, 1)
        norm(1, 0, 1)
        attn_raw(1, 1, fillers=op_fillers(0, 8))
        norm(1, 1, 1)
        attn_raw(2, 1, fillers=op_fillers(1, 12))
        norm(1, 2, 1)
        attn_raw(3, 1, fillers=op_fillers(2, 16))
        norm(1, 3, 1)
        for lt in range(12, 16):
            for nh in range(2):
                op_tile(lt, nh)

        psy_cm.__exit__(None, None, None)
        pso_cm.__exit__(None, None, None)
        pss_cm.__exit__(None, None, None)

    _pin_act_table(nc)
    nc.compile()
    return nc


def _pin_act_table(nc):
    """Make every ACT function resolve to the one table set that holds all
    of {exp, ln, copy, identity}, so the kernel pays a single table load
    instead of thrashing between exp_and_others and natural_log."""
    import types

    import bass_rust as _bass_rust
    from concourse import hw_specs as _hw

    def patched(self):
        has_activation = any(
            isinstance(i, mybir.InstActivation)
            for b in self.main_func.blocks
            for i in b.instructions
        )
        if not has_activation:
            return
        tables = list(_hw.get_activation_tables(self.m.arch).items())
        ours = {AF.Exp, AF.Ln, AF.Copy, AF.Identity, AF.MemsetZero}
        tables = [
            (name, fns if name == "natural_log_exp_and_others"
             else fns - ours)
            for name, fns in tables
        ]
        _bass_rust.insert_act_table_loads(self, tables)

    nc.insert_act_table_loads = types.MethodType(patched, nc)


def vext(vt, pair, hl):
    """lhsT slice of the extended-V tile for (pair, local head hl)."""
    base = VSTR * pair
    if hl == 0:
        return vt[:, base : base + 65]          # M=65: V at 0-63, r at 64
    return vt[:, base + 65 : base + VSTR]       # M=128: ones@32, V at 64-127


def mm_pv(nc, out_ps, hl, c0, lhsT, rhs, start, stop):
    if hl == 0:
        out = out_ps[0:65, c0:QTW]
    else:
        out = out_ps[:, c0:QTW]
    nc.tensor.matmul(out, lhsT, rhs, start=start, stop=stop)


def make_maskT():
    # additive mask: want  (maskT.T @ I)[k, c] = MASKV if k > c else 0
    # => maskT[c, k] = MASKV for k > c: strict upper triangle.
    m = np.zeros((P, P), np.float32)
    m[np.arange(P)[:, None] < np.arange(P)[None, :]] = MASKV
    return m.astype(ml_dtypes.bfloat16)


def make_vpat():
    pat = np.zeros((P, 65), ml_dtypes.bfloat16)
    pat[:, 0] = 1.0   # even-head ones col (tile col 64): r -> partition 64
    pat[:, 33] = 1.0  # odd-head ones col (tile col 97): r -> partition 32
    return pat


def shard_inputs(x, w_qkv, b_qkv, w_out, L=2048):
    """Host-side sharding: core c = (batch c//4, head-group c%4)."""
    x = np.asarray(x, np.float32)
    w_qkv = np.asarray(w_qkv, np.float32)
    b_qkv = np.asarray(b_qkv, np.float32)
    w_out = np.asarray(w_out, np.float32)
    ident = np.eye(P, dtype=ml_dtypes.bfloat16)
    maskT = make_maskT()
    xTs = [np.ascontiguousarray(x[b].T.astype(ml_dtypes.bfloat16))
           for b in range(B)]
    in_maps = []
    for c in range(8):
        b, g = divmod(c, 4)
        qs, ks, vs = 256 * g, D + 256 * g, 2 * D + 256 * g
        wqk = np.ascontiguousarray(
            np.concatenate(
                [w_qkv[:, qs : qs + GD], w_qkv[:, ks : ks + GD]], axis=1
            ).astype(ml_dtypes.bfloat16)
        )
        wv = np.ascontiguousarray(
            w_qkv[:, vs : vs + GD].astype(ml_dtypes.bfloat16))
        wo = np.ascontiguousarray(
            w_out[256 * g : 256 * g + GD, :].astype(ml_dtypes.bfloat16))
        bqk = np.concatenate(
            [b_qkv[qs : qs + GD], b_qkv[ks : ks + GD]]
        ).reshape(2 * GD, 1).astype(np.float32)
        in_maps.append(
            {"xT": xTs[b], "wqk": wqk, "wv": wv, "wo": wo, "bqk": bqk,
             "maskT": maskT, "ident": ident, "vpat": make_vpat()}
        )
    return in_maps


_NC_CACHE = {}


def get_nc(L=2048):
    if L not in _NC_CACHE:
        _NC_CACHE[L] = build_nc(L)
    return _NC_CACHE[L]


def gather(results, b_qkv, w_out, b_out, L=2048):
    fix = (np.asarray(b_qkv, np.float32)[2 * D :] @ np.asarray(w_out, np.float32)
           + np.asarray(b_out, np.float32))
    y = np.zeros((B, L, D), np.float32)
    for c in range(8):
        b = c // 4
        y[b] += results[c]["y"]
    y += fix[None, None, :]
    return y


def kernel(x, w_qkv, b_qkv, w_out, b_out):
    L = x.shape[1]
    nc = get_nc(L)
    in_maps = shard_inputs(x, w_qkv, b_qkv, w_out, L=L)
    res = run_bass_kernel_spmd(nc, in_maps, core_ids=list(range(8)))
    return gather(res.results, b_qkv, w_out, b_out, L=L)


# revision 31
# speedup vs baseline: 1.1771x; 1.0897x over previous
"""Causal multi-head attention (B=2, L=2048, D=1024, H=16) on 8 trn2 cores.

Sharding: DP on batch (2) x TP on heads (4 groups of 4 heads) = 8 cores.
Each core computes, for its (batch b, head-group g):
  - qT/kT = wqk_g^T @ x_b^T            [512, L]   (head dims on partitions)
  - V     = x_b @ wv_g (+ ones cols)   [L, 4*65]  (natural layout, per-head ones
                                                   column so the PV matmul also
                                                   produces softmax denominators)
  - S^T   = K Q^T per (k-block, q-tile), causal-trimmed, both heads of a
            pair row-packed into one concurrent PE pass; additive causal mask
            (-240 pre-scale) accumulated into PSUM by a mask @ I matmul on
            the diagonal squares; ONE exp per k-block on ACT (3-D AP merging
            both heads' trimmed spans)
  - out^T = V_ext^T @ E^T accumulated over k-blocks  -> PSUM
            (partition 64 resp. 32 holds the softmax denominator r)
  - attn_raw^T evicted UNNORMALIZED (fast DVE copies only, so the PSUM
    accumulator ring is short); r rows staged to SBUF
  - normalization out-of-band: r broadcast across partitions via a
    step-0-free-dim SBUF->SBUF DMA, then attn = attn_raw / r elementwise
    on the otherwise-idle GpSimd engine (SBUF-only, which GpSimd requires)
  - y_part = attn @ w_out[rows of g]   [L, 1024]  (row-parallel out-proj)
Host gathers: y_b = sum_g y_part + (b_qkv_v @ w_out + b_out).

Emission is software-pipelined: pair-0 attention for all q-tiles runs right
after the m=0,2 n=0 projections it first needs; the remaining projection
tiles are emitted between attention windows sized so the PE stream stays
dense while ACT grinds exp; out-projection tiles pop into pair-1 windows.
"""

import sys
from contextlib import ExitStack

if "/opt/trn_rl_repo" not in sys.path:
    sys.path.insert(0, "/opt/trn_rl_repo")

import ml_dtypes
import numpy as np

import concourse.bass as bass
import concourse.mybir as mybir
import concourse.tile as tile
from concourse import bacc
from concourse.bass import ts
from concourse.bass_utils import run_bass_kernel_spmd

F32 = mybir.dt.float32
BF16 = mybir.dt.bfloat16
AF = mybir.ActivationFunctionType
OP = mybir.AluOpType

B, D, H = 2, 1024, 16
HD = 64           # head dim
NH = 4            # heads per core
GD = NH * HD      # 256 head dims per core
P = 128
QTW = 512         # q-tile width
VSTR = 193        # per-pair stride in the v tile: [V0(64)|1] + [z32|1|z31|V1(64)]
VW = 2 * VSTR     # v tile width (2 pairs)
MASKV = -240.0    # additive causal mask, pre-exp-scale (0.125) -> -30

NORM_GPSIMD = False  # divide on GpSimd (walrus rejects); Ln/Exp ACT + DVE mult


def bcast_ap(row_ap, n_part):
    """[1, N] SBUF AP -> (1, n_part, N) AP replicating the row (step-0 free
    dim), for DMA partition-broadcast."""
    from concourse.ap import AP

    dims = list(row_ap.ap)
    assert dims[0][1] == 1 and len(dims) == 2, dims
    return AP(row_ap.tensor, row_ap.offset,
              [list(dims[0]), [0, n_part], list(dims[1])])


def build_nc(L=2048):
    """Build the per-core Bass program. Same program for all 8 cores (SPMD)."""
    DK = D // P       # 8 contraction chunks
    LT = L // P       # l-tiles
    QT = L // QTW     # q-tiles
    QB = QTW // P     # k-blocks per q-tile (4)

    nc = bacc.Bacc("TRN2", target_bir_lowering=False, debug=False, num_devices=8)

    xT = nc.dram_tensor("xT", [D, L], BF16, kind="ExternalInput").ap()
    wqk = nc.dram_tensor("wqk", [D, 2 * GD], BF16, kind="ExternalInput").ap()
    wv = nc.dram_tensor("wv", [D, GD], BF16, kind="ExternalInput").ap()
    wo = nc.dram_tensor("wo", [GD, D], BF16, kind="ExternalInput").ap()
    bqk = nc.dram_tensor("bqk", [2 * GD, 1], F32, kind="ExternalInput").ap()
    maskT = nc.dram_tensor("maskT", [P, P], BF16, kind="ExternalInput").ap()
    ident = nc.dram_tensor("ident", [P, P], BF16, kind="ExternalInput").ap()
    # ones/zeros filler for the V slots: [1, 0*32, 1, 0*31] per partition
    vpat = nc.dram_tensor("vpat", [P, 65], BF16, kind="ExternalInput").ap()
    y = nc.dram_tensor("y", [L, D], F32, kind="ExternalOutput").ap()

    with tile.TileContext(nc) as tc, ExitStack() as stk:
        # ---------- persistent SBUF pools ----------
        const = stk.enter_context(tc.tile_pool(name="const", bufs=1))
        qk_pool = stk.enter_context(tc.tile_pool(name="qk", bufs=1))
        v_pool = stk.enter_context(tc.tile_pool(name="v", bufs=1))
        attn_pool = stk.enter_context(tc.tile_pool(name="attn", bufs=1))
        wo_pool = stk.enter_context(tc.tile_pool(name="wo", bufs=1))
        xt_pool = stk.enter_context(tc.tile_pool(name="xt", bufs=1))
        wi_pool = stk.enter_context(tc.tile_pool(name="wi", bufs=1))
        rs_pool = stk.enter_context(tc.tile_pool(name="rs", bufs=1))
        e_pool = stk.enter_context(tc.tile_pool(name="e", bufs=4))
        bc_pool = stk.enter_context(tc.tile_pool(name="bc", bufs=2))
        bc0_pool = stk.enter_context(tc.tile_pool(name="bc0", bufs=1))
        rv_pool = stk.enter_context(tc.tile_pool(name="rv", bufs=2))
        y_pool = stk.enter_context(tc.tile_pool(name="ysb", bufs=4))

        xt_sb = [xt_pool.tile([P, L], BF16, tag=f"xt{k}", name=f"xt_sb{k}") for k in range(DK)]
        # merged weight tiles: one DMA each (DMA issue on the sync engine is
        # ~650ns apiece and serial -- few big DMAs, not many small ones)
        wvb = wi_pool.tile([P, DK * GD], BF16, tag="wvb", name="wvb")
        nc.sync.dma_start(
            wvb[:].rearrange("p (k c) -> p k c", k=DK, c=GD),
            wv[:].rearrange("(k p) c -> p k c", k=DK, p=P))
        wqkb = wi_pool.tile([P, DK * 2 * GD], BF16, tag="wqkb", name="wqkb")
        for h in range(2):
            nc.sync.dma_start(
                wqkb[:, h * 4 * 2 * GD : (h + 1) * 4 * 2 * GD].rearrange(
                    "p (k c) -> p k c", k=4, c=2 * GD),
                wqk[h * 512 : (h + 1) * 512, :].rearrange(
                    "(k p) c -> p k c", k=4, p=P))
        for k in range(DK):
            nc.sync.dma_start(xt_sb[k][:], xT[ts(k, P)])

        def wv_k(k):
            return wvb[:, k * GD : (k + 1) * GD]

        def wqk_km(k, m):
            return wqkb[:, k * 2 * GD + m * P : k * 2 * GD + (m + 1) * P]

        bqk_sb = const.tile([P, 4], F32, tag="bqk", name="bqk_sb")
        nc.sync.dma_start(
            bqk_sb[:].rearrange("p (m o) -> p m o", m=4, o=1),
            bqk[:].rearrange("(m p) o -> p m o", m=4, p=P))
        maskT_sb = const.tile([P, P], BF16, tag="maskT", name="maskT_sb")
        nc.sync.dma_start(maskT_sb[:], maskT)
        ident_sb = const.tile([P, P], BF16, tag="ident", name="ident_sb")
        nc.sync.dma_start(ident_sb[:], ident)

        # m-tile 0,1 = qT (head pairs 01, 23); 2,3 = kT
        qk_sb = [qk_pool.tile([P, L], BF16, tag=f"qk{m}", name=f"qk_sb{m}") for m in range(4)]
        vb = v_pool.tile([P, LT * VW], BF16, tag="vb", name="vb")
        attn_sb = [attn_pool.tile([P, L], BF16, tag=f"attn{p}", name=f"attn_sb{p}") for p in range(2)]
        rs_sb = [rs_pool.tile([P, L], BF16, tag=f"rs{p}", name=f"rs_sb{p}") for p in range(2)]
        wob = wo_pool.tile([P, 2 * D], BF16, tag="wob", name="wob")
        nc.sync.dma_start(
            wob[:].rearrange("p (c d) -> p c d", c=2, d=D),
            wo[:].rearrange("(c p) d -> p c d", c=2, p=P))
        # one strided DMA fills every ones/zeros filler slot in the V tile
        from concourse.ap import AP as _AP
        nc.sync.dma_start(
            vb[:].rearrange("p (s c) -> p s c", s=2 * LT, c=VSTR)[:, :, 64:129],
            _AP(vpat.tensor, 0, [[65, P], [0, 2 * LT], [1, 65]]))

        # ---------- PSUM pools (LIFO lifetimes) ----------
        # pss (4 banks): attention scores; also hosts the V accumulators in
        # phase A (4x 256-wide regions per [P, 1024] tile).
        # pso (2 banks): PV accumulators.  psq (2 banks): q/k projection
        # tiles; closed after the pair-0 sweep and replaced by psy (out-proj).
        pss_cm = tc.tile_pool(name="pss", bufs=2, space="PSUM")
        pss = pss_cm.__enter__()
        pso_cm = tc.tile_pool(name="pso", bufs=1, space="PSUM")
        pso = pso_cm.__enter__()
        psq_cm = tc.tile_pool(name="psq", bufs=1, space="PSUM")
        psq = psq_cm.__enter__()

        # ---------- V projection: per n-block of 4 l-tiles ----------
        # Accumulators live in the pss slots, one group per PSUM bank
        # (regions at 512-f32 bank starts), k-outer to ride the x DMA.
        def v_block(n):
            pvt = [pss.tile([P, 2 * QTW], F32, tag="pss", name=f"ps_v{i}")
                   for i in range(2)]
            regs = [pvt[i // 2][:, QTW * (i % 2) : QTW * (i % 2) + GD]
                    for i in range(4)]
            lts = list(range(4 * n, 4 * n + 4))
            for k in range(DK):
                for i, lt in enumerate(lts):
                    nc.tensor.matmul(
                        regs[i],
                        xt_sb[k][:, ts(lt, P)],
                        wv_k(k),
                        start=(k == 0),
                        stop=(k == DK - 1),
                    )
            for i, lt in enumerate(lts):
                vv = vb[:, lt * VW : (lt + 1) * VW].rearrange(
                    "p (a c) -> p a c", a=2, c=VSTR)
                pv = regs[i].rearrange("p (a c) -> p a c", a=2, c=2 * HD)
                if lt % 2 == 0:
                    nc.vector.tensor_copy(vv[:, :, 0:64], pv[:, :, 0:64])
                    nc.vector.tensor_copy(vv[:, :, 129:193], pv[:, :, 64:128])
                else:
                    nc.scalar.copy(vv[:, :, 0:64], pv[:, :, 0:64])
                    nc.scalar.copy(vv[:, :, 129:193], pv[:, :, 64:128])

        # ---------- q/k projection tiles (each: one [P,QTW] n-slice of m) ----
        # qT/kT: [512, L] = wqk^T @ xT, bias added during PSUM eviction.
        qk_done = set()

        def qk_tile(m, n):
            ps = psq.tile([P, QTW], F32, tag=f"psq{(m + n) % 2}", name="ps_q")
            for k in range(DK):
                nc.tensor.matmul(
                    ps[:],
                    wqk_km(k, m),
                    xt_sb[k][:, ts(n, QTW)],
                    start=(k == 0),
                    stop=(k == DK - 1),
                )
            nc.vector.tensor_scalar(
                out=qk_sb[m][:, ts(n, QTW)],
                in0=ps[:],
                scalar1=bqk_sb[:, m : m + 1],
                scalar2=None,
                op0=OP.add,
            )
            qk_done.add((m, n))

        SKEW = 3

        def attn_raw(qt, pair, fillers=()):
            """Attention for (q-tile qt, head-pair pair): scores+mask+exp+PV,
            then a fast UNNORMALIZED eviction (attn_raw + r rows). `fillers`
            is a list of (unit_idx, closure) to emit between units."""
            q_t = qk_sb[pair]
            k_t = qk_sb[2 + pair]
            assert (pair, qt) in qk_done and (2 + pair, qt) in qk_done
            out_ps = [
                pso.tile([P, QTW], F32, tag=f"pso{h}", name=f"ps_o{h}")
                for h in range(2)
            ]
            nblk = QB * qt + QB     # k-blocks for this q-tile
            fill = {}
            for u, clo in fillers:
                fill.setdefault(u % nblk, []).append(clo)

            def front(j):
                sp = pss.tile([P, 2 * QTW], F32, tag="pss", name="ps_s")
                e_t = e_pool.tile([P, 2 * QTW], BF16, tag="e", name="e_t")
                diag = j >= QB * qt
                da = (j - QB * qt) * P if diag else 0
                for hl in range(2):
                    hb = 64 * hl
                    nc.tensor.matmul(
                        sp[:, hl * QTW + da : (hl + 1) * QTW],
                        k_t[hb : hb + 64, ts(j, P)],
                        q_t[hb : hb + 64,
                            qt * QTW + da : (qt + 1) * QTW],
                        start=True, stop=True)
                if diag:
                    # additive causal mask on the diagonal square:
                    # sp[:, da:da+P] += maskT.T @ I  (strict upper = -240)
                    for hl in range(2):
                        nc.tensor.matmul(
                            sp[:, hl * QTW + da : hl * QTW + da + P],
                            maskT_sb[:],
                            ident_sb[:],
                            start=False, stop=True,
                            skip_group_check=True)
                if da == 0:
                    nc.scalar.activation(e_t[:], sp[:], AF.Exp, scale=0.125)
                else:
                    spv = sp[:].rearrange("p (h c) -> p h c", h=2, c=QTW)
                    etv = e_t[:].rearrange("p (h c) -> p h c", h=2, c=QTW)
                    nc.scalar.activation(etv[:, :, da:QTW], spv[:, :, da:QTW],
                                         AF.Exp, scale=0.125)

                def emit_pv(j=j, da=da, e_t=e_t):
                    for hl in range(2):
                        mm_pv(nc, out_ps[hl], hl, da,
                              vext(vb, j, pair, hl),
                              e_t[:, hl * QTW + da : (hl + 1) * QTW],
                              start=(j == 0), stop=(j == nblk - 1))
                return emit_pv

            pend = []
            for j in range(nblk):
                pend.append(front(j))
                for clo in fill.get(j, ()):
                    clo()
                if j >= SKEW:
                    pend[j - SKEW]()
            for j in range(max(0, nblk - SKEW), nblk):
                pend[j]()

            # fast raw eviction: attn_raw (bf16) + r rows -> rs (bf16). The
            # out_ps ring contains only these four DVE copies.
            nc.vector.tensor_copy(attn_sb[pair][0:64, ts(qt, QTW)],
                                  out_ps[0][0:64, :])
            nc.vector.tensor_copy(attn_sb[pair][64:P, ts(qt, QTW)],
                                  out_ps[1][64:P, :])
            nc.vector.tensor_copy(rs_sb[pair][64:65, ts(qt, QTW)],
                                  out_ps[0][64:65, :])
            nc.vector.tensor_copy(rs_sb[pair][32:33, ts(qt, QTW)],
                                  out_ps[1][32:33, :])

        def norm(pair, q0, nqt):
            """Normalize attn_sb[pair][:, q0*QTW : (q0+nqt)*QTW] by its r."""
            w = nqt * QTW
            sl = slice(q0 * QTW, q0 * QTW + w)
            if NORM_GPSIMD:
                if nqt > 1:
                    bc = bc0_pool.tile([P, w], BF16, tag="bc0", name="bc0_t")
                else:
                    bc = bc_pool.tile([P, QTW], BF16, tag="bc", name="bc_t")
                nc.sync.dma_start(bc[0:64, 0:w],
                                  bcast_ap(rs_sb[pair][64:65, sl], 64))
                nc.sync.dma_start(bc[64:P, 0:w],
                                  bcast_ap(rs_sb[pair][32:33, sl], 64))
                nc.gpsimd.tensor_tensor(
                    out=attn_sb[pair][:, sl],
                    in0=attn_sb[pair][:, sl],
                    in1=bc[:, 0:w], op=OP.divide)
            else:
                rv = rv_pool.tile([P, w], F32, tag=f"rv{w}", name="rv_t")
                if nqt > 1:
                    bc = bc0_pool.tile([P, w], F32, tag="bc0", name="bc0_t")
                else:
                    bc = bc_pool.tile([P, QTW], F32, tag="bc", name="bc_t")
                nc.scalar.activation(rv[64:65, 0:w], rs_sb[pair][64:65, sl],
                                     AF.Ln)
                nc.scalar.activation(rv[32:33, 0:w], rs_sb[pair][32:33, sl],
                                     AF.Ln)
                nc.scalar.activation(rv[64:65, 0:w], rv[64:65, 0:w], AF.Exp,
                                     scale=-1.0)
                nc.scalar.activation(rv[32:33, 0:w], rv[32:33, 0:w], AF.Exp,
                                     scale=-1.0)
                nc.sync.dma_start(bc[0:64, 0:w], bcast_ap(rv[64:65, 0:w], 64))
                nc.sync.dma_start(bc[64:P, 0:w], bcast_ap(rv[32:33, 0:w], 64))
                nc.vector.tensor_tensor(
                    out=attn_sb[pair][0:64, sl],
                    in0=attn_sb[pair][0:64, sl], in1=bc[0:64, 0:w],
                    op=OP.mult)
                nc.vector.tensor_tensor(
                    out=attn_sb[pair][64:P, sl],
                    in0=attn_sb[pair][64:P, sl], in1=bc[64:P, 0:w],
                    op=OP.mult)

        # ---------- pair-0 sweep (V + projection tiles interleaved) ----------
        v_block(0)
        qk_tile(0, 0)
        qk_tile(2, 0)
        attn_raw(0, 0)
        v_block(1)
        qk_tile(0, 1)
        qk_tile(2, 1)
        attn_raw(1, 0, fillers=[
            (2, lambda: qk_tile(1, 0)), (5, lambda: qk_tile(3, 0))])
        v_block(2)
        qk_tile(0, 2)
        qk_tile(2, 2)
        attn_raw(2, 0, fillers=[
            (2, lambda: qk_tile(1, 1)), (6, lambda: qk_tile(3, 1))])
        v_block(3)
        qk_tile(0, 3)
        qk_tile(2, 3)
        attn_raw(3, 0, fillers=[
            (2, lambda: qk_tile(1, 2)), (5, lambda: qk_tile(3, 2)),
            (8, lambda: qk_tile(1, 3)), (11, lambda: qk_tile(3, 3))])
        for qt in range(QT):
            norm(0, qt, 1)

        # q/k psum space becomes the out-projection pool
        psq_cm.__exit__(None, None, None)
        psy_cm = tc.tile_pool(name="psy", bufs=2, space="PSUM")
        psy = psy_cm.__enter__()

        # ---------- pair-1 sweep (out-projection tiles interleaved) ------
        def op_tile(lt):
            pss_ = [psy.tile([P, QTW], F32, tag="psy", name="ps_y")
                    for _ in range(2)]
            for nh in range(2):
                for c in range(2):
                    nc.tensor.matmul(
                        pss_[nh][:],
                        attn_sb[c][:, ts(lt, P)],
                        wob[:, c * D + nh * QTW : c * D + (nh + 1) * QTW],
                        start=(c == 0),
                        stop=(c == 1),
                    )
            yt = y_pool.tile([P, D], F32, tag="y", name="y_t")
            for nh in range(2):
                if lt >= 12:       # tail tiles: ACT is idle by then
                    nc.scalar.copy(yt[:, ts(nh, QTW)], pss_[nh][:])
                else:
                    nc.vector.tensor_copy(yt[:, ts(nh, QTW)], pss_[nh][:])
            nc.sync.dma_start(y[ts(lt, P), :], yt[:])

        def op_fillers(qt, units):
            out = []
            tiles = list(range(4 * qt, 4 * qt + 4))
            step = max(1, units // len(tiles))
            for i, lt in enumerate(tiles):
                out.append((1 + i * step, lambda lt=lt: op_tile(lt)))
            return out

        attn_raw(0, 1)
        norm(1, 0, 1)
        attn_raw(1, 1, fillers=op_fillers(0, 8))
        norm(1, 1, 1)
        attn_raw(2, 1, fillers=op_fillers(1, 12))
        norm(1, 2, 1)
        attn_raw(3, 1, fillers=op_fillers(2, 16))
        norm(1, 3, 1)
        for lt in range(12, 16):
            op_tile(lt)

        psy_cm.__exit__(None, None, None)
        pso_cm.__exit__(None, None, None)
        pss_cm.__exit__(None, None, None)

    _pin_act_table(nc)
    nc.compile()
    return nc


def _pin_act_table(nc):
    """Make every ACT function resolve to the one table set that holds all
    of {exp, ln, copy, identity}, so the kernel pays a single table load
    instead of thrashing between exp_and_others and natural_log."""
    import types

    import bass_rust as _bass_rust
    from concourse import hw_specs as _hw

    def patched(self):
        has_activation = any(
            isinstance(i, mybir.InstActivation)
            for b in self.main_func.blocks
            for i in b.instructions
        )
        if not has_activation:
            return
        tables = list(_hw.get_activation_tables(self.m.arch).items())
        ours = {AF.Exp, AF.Ln, AF.Copy, AF.Identity, AF.MemsetZero}
        tables = [
            (name, fns if name == "natural_log_exp_and_others"
             else fns - ours)
            for name, fns in tables
        ]
        _bass_rust.insert_act_table_loads(self, tables)

    nc.insert_act_table_loads = types.MethodType(patched, nc)


def vext(vb, lt, pair, hl):
    """lhsT slice of the extended-V tile for (l-tile lt, pair, local head)."""
    base = lt * VW + VSTR * pair
    if hl == 0:
        return vb[:, base : base + 65]          # M=65: V at 0-63, r at 64
    return vb[:, base + 65 : base + VSTR]       # M=128: ones@32, V at 64-127


def mm_pv(nc, out_ps, hl, c0, lhsT, rhs, start, stop):
    if hl == 0:
        out = out_ps[0:65, c0:QTW]
    else:
        out = out_ps[:, c0:QTW]
    nc.tensor.matmul(out, lhsT, rhs, start=start, stop=stop)


def make_maskT():
    # additive mask: want  (maskT.T @ I)[k, c] = MASKV if k > c else 0
    # => maskT[c, k] = MASKV for k > c: strict upper triangle.
    m = np.zeros((P, P), np.float32)
    m[np.arange(P)[:, None] < np.arange(P)[None, :]] = MASKV
    return m.astype(ml_dtypes.bfloat16)


def make_vpat():
    pat = np.zeros((P, 65), ml_dtypes.bfloat16)
    pat[:, 0] = 1.0   # even-head ones col (tile col 64): r -> partition 64
    pat[:, 33] = 1.0  # odd-head ones col (tile col 97): r -> partition 32
    return pat


def shard_inputs(x, w_qkv, b_qkv, w_out, L=2048):
    """Host-side sharding: core c = (batch c//4, head-group c%4)."""
    x = np.asarray(x, np.float32)
    w_qkv = np.asarray(w_qkv, np.float32)
    b_qkv = np.asarray(b_qkv, np.float32)
    w_out = np.asarray(w_out, np.float32)
    ident = np.eye(P, dtype=ml_dtypes.bfloat16)
    maskT = make_maskT()
    xTs = [np.ascontiguousarray(x[b].T.astype(ml_dtypes.bfloat16))
           for b in range(B)]
    in_maps = []
    for c in range(8):
        b, g = divmod(c, 4)
        qs, ks, vs = 256 * g, D + 256 * g, 2 * D + 256 * g
        wqk = np.ascontiguousarray(
            np.concatenate(
                [w_qkv[:, qs : qs + GD], w_qkv[:, ks : ks + GD]], axis=1
            ).astype(ml_dtypes.bfloat16)
        )
        wv = np.ascontiguousarray(
            w_qkv[:, vs : vs + GD].astype(ml_dtypes.bfloat16))
        wo = np.ascontiguousarray(
            w_out[256 * g : 256 * g + GD, :].astype(ml_dtypes.bfloat16))
        bqk = np.concatenate(
            [b_qkv[qs : qs + GD], b_qkv[ks : ks + GD]]
        ).reshape(2 * GD, 1).astype(np.float32)
        in_maps.append(
            {"xT": xTs[b], "wqk": wqk, "wv": wv, "wo": wo, "bqk": bqk,
             "maskT": maskT, "ident": ident, "vpat": make_vpat()}
        )
    return in_maps


_NC_CACHE = {}


def get_nc(L=2048):
    if L not in _NC_CACHE:
        _NC_CACHE[L] = build_nc(L)
    return _NC_CACHE[L]


def gather(results, b_qkv, w_out, b_out, L=2048):
    fix = (np.asarray(b_qkv, np.float32)[2 * D :] @ np.asarray(w_out, np.float32)
           + np.asarray(b_out, np.float32))
    y = np.zeros((B, L, D), np.float32)
    for c in range(8):
        b = c // 4
        y[b] += results[c]["y"]
    y += fix[None, None, :]
    return y


def kernel(x, w_qkv, b_qkv, w_out, b_out):
    L = x.shape[1]
    nc = get_nc(L)
    in_maps = shard_inputs(x, w_qkv, b_qkv, w_out, L=L)
    res = run_bass_kernel_spmd(nc, in_maps, core_ids=list(range(8)))
    return gather(res.results, b_qkv, w_out, b_out, L=L)


# revision 35
# speedup vs baseline: 1.2255x; 1.0411x over previous
"""Causal multi-head attention (B=2, L=2048, D=1024, H=16) on 8 trn2 cores.

Sharding: DP on batch (2) x TP on heads (4 groups of 4 heads) = 8 cores.
Each core computes, for its (batch b, head-group g):
  - qT/kT = wqk_g^T @ x_b^T            [512, L]   (head dims on partitions)
  - V     = x_b @ wv_g (+ ones cols)   [L, 4*65]  (natural layout, per-head ones
                                                   column so the PV matmul also
                                                   produces softmax denominators)
  - S^T   = K Q^T per (k-block, q-tile), causal-trimmed, both heads of a
            pair row-packed into one concurrent PE pass; additive causal mask
            (-240 pre-scale) accumulated into PSUM by a mask @ I matmul on
            the diagonal squares; ONE exp per k-block on ACT (3-D AP merging
            both heads' trimmed spans)
  - out^T = V_ext^T @ E^T accumulated over k-blocks  -> PSUM
            (partition 64 resp. 32 holds the softmax denominator r)
  - attn_raw^T evicted UNNORMALIZED (fast DVE copies only, so the PSUM
    accumulator ring is short); r rows staged to SBUF
  - normalization out-of-band: r broadcast across partitions via a
    step-0-free-dim SBUF->SBUF DMA, then attn = attn_raw / r elementwise
    on the otherwise-idle GpSimd engine (SBUF-only, which GpSimd requires)
  - y_part = attn @ w_out[rows of g]   [L, 1024]  (row-parallel out-proj)
Host gathers: y_b = sum_g y_part + (b_qkv_v @ w_out + b_out).

Emission is software-pipelined: pair-0 attention for all q-tiles runs right
after the m=0,2 n=0 projections it first needs; the remaining projection
tiles are emitted between attention windows sized so the PE stream stays
dense while ACT grinds exp; out-projection tiles pop into pair-1 windows.
"""

import sys
from contextlib import ExitStack

if "/opt/trn_rl_repo" not in sys.path:
    sys.path.insert(0, "/opt/trn_rl_repo")

import ml_dtypes
import numpy as np

import concourse.bass as bass
import concourse.mybir as mybir
import concourse.tile as tile
from concourse import bacc
from concourse.bass import ts
from concourse.bass_utils import run_bass_kernel_spmd

F32 = mybir.dt.float32
BF16 = mybir.dt.bfloat16
AF = mybir.ActivationFunctionType
OP = mybir.AluOpType

B, D, H = 2, 1024, 16
HD = 64           # head dim
NH = 4            # heads per core
GD = NH * HD      # 256 head dims per core
P = 128
QTW = 512         # q-tile width
VSTR = 193        # per-pair stride in the v tile: [V0(64)|1] + [z32|1|z31|V1(64)]
VW = 2 * VSTR     # v tile width (2 pairs)
MASKV = -240.0    # additive causal mask, pre-exp-scale (0.125) -> -30

NORM_GPSIMD = False  # divide on GpSimd (walrus rejects); Ln/Exp ACT + DVE mult


def bcast_ap(row_ap, n_part):
    """[1, N] SBUF AP -> (1, n_part, N) AP replicating the row (step-0 free
    dim), for DMA partition-broadcast."""
    from concourse.ap import AP

    dims = list(row_ap.ap)
    assert dims[0][1] == 1 and len(dims) == 2, dims
    return AP(row_ap.tensor, row_ap.offset,
              [list(dims[0]), [0, n_part], list(dims[1])])


def build_nc(L=2048):
    """Build the per-core Bass program. Same program for all 8 cores (SPMD)."""
    DK = D // P       # 8 contraction chunks
    LT = L // P       # l-tiles
    QT = L // QTW     # q-tiles
    QB = QTW // P     # k-blocks per q-tile (4)

    nc = bacc.Bacc("TRN2", target_bir_lowering=False, debug=False, num_devices=8)

    xT = nc.dram_tensor("xT", [D, L], BF16, kind="ExternalInput").ap()
    wqk = nc.dram_tensor("wqk", [D, 2 * GD], BF16, kind="ExternalInput").ap()
    wv = nc.dram_tensor("wv", [D, GD], BF16, kind="ExternalInput").ap()
    wo = nc.dram_tensor("wo", [GD, D], BF16, kind="ExternalInput").ap()
    bqk = nc.dram_tensor("bqk", [2 * GD, 1], F32, kind="ExternalInput").ap()
    maskT = nc.dram_tensor("maskT", [P, P], BF16, kind="ExternalInput").ap()
    ident = nc.dram_tensor("ident", [P, P], BF16, kind="ExternalInput").ap()
    # ones/zeros filler for the V slots: [1, 0*32, 1, 0*31] per partition
    vpat = nc.dram_tensor("vpat", [P, 65], BF16, kind="ExternalInput").ap()
    y = nc.dram_tensor("y", [L, D], F32, kind="ExternalOutput").ap()

    with tile.TileContext(nc) as tc, ExitStack() as stk:
        # ---------- persistent SBUF pools ----------
        const = stk.enter_context(tc.tile_pool(name="const", bufs=1))
        qk_pool = stk.enter_context(tc.tile_pool(name="qk", bufs=1))
        v_pool = stk.enter_context(tc.tile_pool(name="v", bufs=1))
        attn_pool = stk.enter_context(tc.tile_pool(name="attn", bufs=1))
        wo_pool = stk.enter_context(tc.tile_pool(name="wo", bufs=1))
        xt_pool = stk.enter_context(tc.tile_pool(name="xt", bufs=1))
        wi_pool = stk.enter_context(tc.tile_pool(name="wi", bufs=1))
        rs_pool = stk.enter_context(tc.tile_pool(name="rs", bufs=1))
        e_pool = stk.enter_context(tc.tile_pool(name="e", bufs=4))
        bc_pool = stk.enter_context(tc.tile_pool(name="bc", bufs=2))
        bc0_pool = stk.enter_context(tc.tile_pool(name="bc0", bufs=1))
        rv_pool = stk.enter_context(tc.tile_pool(name="rv", bufs=2))
        y_pool = stk.enter_context(tc.tile_pool(name="ysb", bufs=4))

        xt_sb = [xt_pool.tile([P, L], BF16, tag=f"xt{k}", name=f"xt_sb{k}") for k in range(DK)]
        # merged weight tiles: one DMA each (DMA issue on the sync engine is
        # ~650ns apiece and serial -- few big DMAs, not many small ones)
        wvb = wi_pool.tile([P, DK * GD], BF16, tag="wvb", name="wvb")
        nc.sync.dma_start(
            wvb[:].rearrange("p (k c) -> p k c", k=DK, c=GD),
            wv[:].rearrange("(k p) c -> p k c", k=DK, p=P))
        wqkb = wi_pool.tile([P, DK * 2 * GD], BF16, tag="wqkb", name="wqkb")
        for h in range(2):
            nc.sync.dma_start(
                wqkb[:, h * 4 * 2 * GD : (h + 1) * 4 * 2 * GD].rearrange(
                    "p (k c) -> p k c", k=4, c=2 * GD),
                wqk[h * 512 : (h + 1) * 512, :].rearrange(
                    "(k p) c -> p k c", k=4, p=P))
        for k in range(DK):
            nc.sync.dma_start(xt_sb[k][:], xT[ts(k, P)])

        def wv_k(k):
            return wvb[:, k * GD : (k + 1) * GD]

        def wqk_km(k, m):
            return wqkb[:, k * 2 * GD + m * P : k * 2 * GD + (m + 1) * P]

        bqk_sb = const.tile([P, 4], F32, tag="bqk", name="bqk_sb")
        nc.sync.dma_start(
            bqk_sb[:].rearrange("p (m o) -> p m o", m=4, o=1),
            bqk[:].rearrange("(m p) o -> p m o", m=4, p=P))
        maskT_sb = const.tile([P, P], BF16, tag="maskT", name="maskT_sb")
        nc.sync.dma_start(maskT_sb[:], maskT)
        ident_sb = const.tile([P, P], BF16, tag="ident", name="ident_sb")
        nc.sync.dma_start(ident_sb[:], ident)

        # m-tile 0,1 = qT (head pairs 01, 23); 2,3 = kT
        qk_sb = [qk_pool.tile([P, L], BF16, tag=f"qk{m}", name=f"qk_sb{m}") for m in range(4)]
        vb = v_pool.tile([P, LT * VW], BF16, tag="vb", name="vb")
        attn_sb = [attn_pool.tile([P, L], BF16, tag=f"attn{p}", name=f"attn_sb{p}") for p in range(2)]
        rs_sb = [rs_pool.tile([P, L], BF16, tag=f"rs{p}", name=f"rs_sb{p}") for p in range(2)]
        wob = wo_pool.tile([P, 2 * D], BF16, tag="wob", name="wob")
        nc.sync.dma_start(
            wob[:].rearrange("p (c d) -> p c d", c=2, d=D),
            wo[:].rearrange("(c p) d -> p c d", c=2, p=P))
        # one strided DMA fills every ones/zeros filler slot in the V tile
        from concourse.ap import AP as _AP
        nc.sync.dma_start(
            vb[:].rearrange("p (s c) -> p s c", s=2 * LT, c=VSTR)[:, :, 64:129],
            _AP(vpat.tensor, 0, [[65, P], [0, 2 * LT], [1, 65]]))

        # ---------- PSUM pools (LIFO lifetimes) ----------
        # pss (4 banks): attention scores; also hosts the V accumulators in
        # phase A (4x 256-wide regions per [P, 1024] tile).
        # pso (2 banks): PV accumulators.  psq (2 banks): q/k projection
        # tiles; closed after the pair-0 sweep and replaced by psy (out-proj).
        pss_cm = tc.tile_pool(name="pss", bufs=2, space="PSUM")
        pss = pss_cm.__enter__()
        pso_cm = tc.tile_pool(name="pso", bufs=1, space="PSUM")
        pso = pso_cm.__enter__()
        psq_cm = tc.tile_pool(name="psq", bufs=1, space="PSUM")
        psq = psq_cm.__enter__()

        # ---------- V projection: per n-block of 4 l-tiles ----------
        # Accumulators live in the pss slots, one group per PSUM bank
        # (regions at 512-f32 bank starts), k-outer to ride the x DMA.
        def v_block(n):
            pvt = [pss.tile([P, 2 * QTW], F32, tag="pss", name=f"ps_v{i}")
                   for i in range(2)]
            regs = [pvt[i // 2][:, QTW * (i % 2) : QTW * (i % 2) + GD]
                    for i in range(4)]
            lts = list(range(4 * n, 4 * n + 4))
            for k in range(DK):
                for i, lt in enumerate(lts):
                    nc.tensor.matmul(
                        regs[i],
                        xt_sb[k][:, ts(lt, P)],
                        wv_k(k),
                        start=(k == 0),
                        stop=(k == DK - 1),
                    )
            for i, lt in enumerate(lts):
                vv = vb[:, lt * VW : (lt + 1) * VW].rearrange(
                    "p (a c) -> p a c", a=2, c=VSTR)
                pv = regs[i].rearrange("p (a c) -> p a c", a=2, c=2 * HD)
                if lt % 2 == 0:
                    nc.vector.tensor_copy(vv[:, :, 0:64], pv[:, :, 0:64])
                    nc.vector.tensor_copy(vv[:, :, 129:193], pv[:, :, 64:128])
                else:
                    nc.scalar.copy(vv[:, :, 0:64], pv[:, :, 0:64])
                    nc.scalar.copy(vv[:, :, 129:193], pv[:, :, 64:128])

        # ---------- q/k projection tiles (each: one [P,QTW] n-slice of m) ----
        # qT/kT: [512, L] = wqk^T @ xT, bias added during PSUM eviction.
        qk_done = set()

        def qk_tile(m, n):
            ps = psq.tile([P, QTW], F32, tag=f"psq{(m + n) % 2}", name="ps_q")
            for k in range(DK):
                nc.tensor.matmul(
                    ps[:],
                    wqk_km(k, m),
                    xt_sb[k][:, ts(n, QTW)],
                    start=(k == 0),
                    stop=(k == DK - 1),
                )
            nc.vector.tensor_scalar(
                out=qk_sb[m][:, ts(n, QTW)],
                in0=ps[:],
                scalar1=bqk_sb[:, m : m + 1],
                scalar2=None,
                op0=OP.add,
            )
            qk_done.add((m, n))

        SKEW = 3

        def attn_raw(qt, pair, fillers=()):
            """Attention for (q-tile qt, head-pair pair): scores+mask+exp+PV,
            then a fast UNNORMALIZED eviction (attn_raw + r rows). `fillers`
            is a list of (unit_idx, closure) to emit between units."""
            q_t = qk_sb[pair]
            k_t = qk_sb[2 + pair]
            assert (pair, qt) in qk_done and (2 + pair, qt) in qk_done
            out_ps = [
                pso.tile([P, QTW], F32, tag=f"pso{h}", name=f"ps_o{h}")
                for h in range(2)
            ]
            nblk = QB * qt + QB     # k-blocks for this q-tile
            fill = {}
            for u, clo in fillers:
                fill.setdefault(u % nblk, []).append(clo)

            def front(j):
                sp = pss.tile([P, 2 * QTW], F32, tag="pss", name="ps_s")
                e_t = e_pool.tile([P, 2 * QTW], BF16, tag="e", name="e_t")
                diag = j >= QB * qt
                da = (j - QB * qt) * P if diag else 0
                for hl in range(2):
                    hb = 64 * hl
                    nc.tensor.matmul(
                        sp[:, hl * QTW + da : (hl + 1) * QTW],
                        k_t[hb : hb + 64, ts(j, P)],
                        q_t[hb : hb + 64,
                            qt * QTW + da : (qt + 1) * QTW],
                        start=True, stop=True)
                if diag:
                    # additive causal mask on the diagonal square:
                    # sp[:, da:da+P] += maskT.T @ I  (strict upper = -240)
                    for hl in range(2):
                        nc.tensor.matmul(
                            sp[:, hl * QTW + da : hl * QTW + da + P],
                            maskT_sb[:],
                            ident_sb[:],
                            start=False, stop=True,
                            skip_group_check=True)
                if da == 0:
                    nc.scalar.activation(e_t[:], sp[:], AF.Exp, scale=0.125)
                else:
                    spv = sp[:].rearrange("p (h c) -> p h c", h=2, c=QTW)
                    etv = e_t[:].rearrange("p (h c) -> p h c", h=2, c=QTW)
                    nc.scalar.activation(etv[:, :, da:QTW], spv[:, :, da:QTW],
                                         AF.Exp, scale=0.125)

                def emit_pv(j=j, da=da, e_t=e_t):
                    for hl in range(2):
                        mm_pv(nc, out_ps[hl], hl, da,
                              vext(vb, j, pair, hl),
                              e_t[:, hl * QTW + da : (hl + 1) * QTW],
                              start=(j == 0), stop=(j == nblk - 1))
                return emit_pv

            pend = []
            for j in range(nblk):
                pend.append(front(j))
                for clo in fill.get(j, ()):
                    clo()
                if j >= SKEW:
                    pend[j - SKEW]()
            for j in range(max(0, nblk - SKEW), nblk):
                pend[j]()

            # fast raw eviction: attn_raw (bf16) + r rows -> rs (bf16). The
            # out_ps ring contains only these four DVE copies.
            nc.vector.tensor_copy(attn_sb[pair][0:64, ts(qt, QTW)],
                                  out_ps[0][0:64, :])
            nc.vector.tensor_copy(attn_sb[pair][64:P, ts(qt, QTW)],
                                  out_ps[1][64:P, :])
            nc.vector.tensor_copy(rs_sb[pair][64:65, ts(qt, QTW)],
                                  out_ps[0][64:65, :])
            nc.vector.tensor_copy(rs_sb[pair][32:33, ts(qt, QTW)],
                                  out_ps[1][32:33, :])

        def two_rows(row32_ap):
            """[1, w] AP at partition 32 -> [2, w] AP over partitions 32, 64."""
            from concourse.ap import AP
            dims = [list(d) for d in row32_ap.ap]
            assert dims[0][1] == 1
            return AP(row32_ap.tensor, row32_ap.offset,
                      [[32, 2]] + dims[1:])

        def norm(pair, q0, nqt=1):
            """Normalize attn_sb[pair][:, q0*QTW:...] by its r (one q-tile)."""
            assert nqt == 1
            w = QTW
            sl = slice(q0 * QTW, q0 * QTW + w)
            rv = rv_pool.tile([P, w], F32, tag="rv", name="rv_t")
            bc = bc_pool.tile([P, QTW], F32, tag="bc", name="bc_t")
            nc.scalar.activation(rv[64:65, 0:w], rs_sb[pair][64:65, sl], AF.Ln)
            nc.scalar.activation(rv[32:33, 0:w], rs_sb[pair][32:33, sl], AF.Ln)
            nc.scalar.activation(rv[64:65, 0:w], rv[64:65, 0:w], AF.Exp,
                                 scale=-1.0)
            nc.scalar.activation(rv[32:33, 0:w], rv[32:33, 0:w], AF.Exp,
                                 scale=-1.0)
            nc.sync.dma_start(bc[0:64, 0:w], bcast_ap(rv[64:65, 0:w], 64))
            nc.sync.dma_start(bc[64:P, 0:w], bcast_ap(rv[32:33, 0:w], 64))
            nc.vector.tensor_tensor(
                out=attn_sb[pair][:, sl],
                in0=attn_sb[pair][:, sl], in1=bc[:, 0:w], op=OP.mult)

        # ---------- pair-0 sweep (V + projection tiles interleaved) ----------
        v_block(0)
        qk_tile(0, 0)
        qk_tile(2, 0)
        attn_raw(0, 0)
        v_block(1)
        qk_tile(0, 1)
        qk_tile(2, 1)
        attn_raw(1, 0, fillers=[
            (2, lambda: qk_tile(1, 0)), (5, lambda: qk_tile(3, 0))])
        v_block(2)
        qk_tile(0, 2)
        qk_tile(2, 2)
        attn_raw(2, 0, fillers=[
            (2, lambda: qk_tile(1, 1)), (6, lambda: qk_tile(3, 1))])
        v_block(3)
        qk_tile(0, 3)
        qk_tile(2, 3)
        attn_raw(3, 0, fillers=[
            (2, lambda: qk_tile(1, 2)), (5, lambda: qk_tile(3, 2)),
            (8, lambda: qk_tile(1, 3)), (11, lambda: qk_tile(3, 3))])

        # q/k psum space becomes the out-projection pool
        psq_cm.__exit__(None, None, None)
        psy_cm = tc.tile_pool(name="psy", bufs=2, space="PSUM")
        psy = psy_cm.__enter__()

        # ---------- pair-1 sweep (out-projection tiles interleaved) ------
        def op_tile(lt):
            pss_ = [psy.tile([P, QTW], F32, tag="psy", name="ps_y")
                    for _ in range(2)]
            for nh in range(2):
                for c in range(2):
                    nc.tensor.matmul(
                        pss_[nh][:],
                        attn_sb[c][:, ts(lt, P)],
                        wob[:, c * D + nh * QTW : c * D + (nh + 1) * QTW],
                        start=(c == 0),
                        stop=(c == 1),
                    )
            yt = y_pool.tile([P, D], F32, tag="y", name="y_t")
            for nh in range(2):
                if lt >= 12:       # tail tiles: ACT is idle by then
                    nc.scalar.copy(yt[:, ts(nh, QTW)], pss_[nh][:])
                else:
                    nc.vector.tensor_copy(yt[:, ts(nh, QTW)], pss_[nh][:])
            nc.sync.dma_start(y[ts(lt, P), :], yt[:])

        def op_fillers(qt, units):
            out = []
            tiles = list(range(4 * qt, 4 * qt + 4))
            step = max(1, units // len(tiles))
            for i, lt in enumerate(tiles):
                out.append((1 + i * step, lambda lt=lt: op_tile(lt)))
            return out

        attn_raw(0, 1, fillers=[
            (0, lambda: norm(0, 0)), (1, lambda: norm(0, 1)),
            (2, lambda: norm(0, 2)), (3, lambda: norm(0, 3))])
        norm(1, 0)
        attn_raw(1, 1, fillers=op_fillers(0, 8))
        norm(1, 1)
        attn_raw(2, 1, fillers=op_fillers(1, 12))
        norm(1, 2)
        attn_raw(3, 1, fillers=op_fillers(2, 16))
        norm(1, 3)
        for lt in range(12, 16):
            op_tile(lt)

        psy_cm.__exit__(None, None, None)
        pso_cm.__exit__(None, None, None)
        pss_cm.__exit__(None, None, None)

    _pin_act_table(nc)
    nc.compile()
    return nc


def _pin_act_table(nc):
    """Make every ACT function resolve to the one table set that holds all
    of {exp, ln, copy, identity}, so the kernel pays a single table load
    instead of thrashing between exp_and_others and natural_log."""
    import types

    import bass_rust as _bass_rust
    from concourse import hw_specs as _hw

    def patched(self):
        has_activation = any(
            isinstance(i, mybir.InstActivation)
            for b in self.main_func.blocks
            for i in b.instructions
        )
        if not has_activation:
            return
        tables = list(_hw.get_activation_tables(self.m.arch).items())
        ours = {AF.Exp, AF.Ln, AF.Copy, AF.Identity, AF.MemsetZero}
        tables = [
            (name, fns if name == "natural_log_exp_and_others"
             else fns - ours)
            for name, fns in tables
        ]
        _bass_rust.insert_act_table_loads(self, tables)

    nc.insert_act_table_loads = types.MethodType(patched, nc)


def vext(vb, lt, pair, hl):
    """lhsT slice of the extended-V tile for (l-tile lt, pair, local head)."""
    base = lt * VW + VSTR * pair
    if hl == 0:
        return vb[:, base : base + 65]          # M=65: V at 0-63, r at 64
    return vb[:, base + 65 : base + VSTR]       # M=128: ones@32, V at 64-127


def mm_pv(nc, out_ps, hl, c0, lhsT, rhs, start, stop):
    if hl == 0:
        out = out_ps[0:65, c0:QTW]
    else:
        out = out_ps[:, c0:QTW]
    nc.tensor.matmul(out, lhsT, rhs, start=start, stop=stop)


def make_maskT():
    # additive mask: want  (maskT.T @ I)[k, c] = MASKV if k > c else 0
    # => maskT[c, k] = MASKV for k > c: strict upper triangle.
    m = np.zeros((P, P), np.float32)
    m[np.arange(P)[:, None] < np.arange(P)[None, :]] = MASKV
    return m.astype(ml_dtypes.bfloat16)


def make_vpat():
    pat = np.zeros((P, 65), ml_dtypes.bfloat16)
    pat[:, 0] = 1.0   # even-head ones col (tile col 64): r -> partition 64
    pat[:, 33] = 1.0  # odd-head ones col (tile col 97): r -> partition 32
    return pat


def shard_inputs(x, w_qkv, b_qkv, w_out, L=2048):
    """Host-side sharding: core c = (batch c//4, head-group c%4)."""
    x = np.asarray(x, np.float32)
    w_qkv = np.asarray(w_qkv, np.float32)
    b_qkv = np.asarray(b_qkv, np.float32)
    w_out = np.asarray(w_out, np.float32)
    ident = np.eye(P, dtype=ml_dtypes.bfloat16)
    maskT = make_maskT()
    xTs = [np.ascontiguousarray(x[b].T.astype(ml_dtypes.bfloat16))
           for b in range(B)]
    in_maps = []
    for c in range(8):
        b, g = divmod(c, 4)
        qs, ks, vs = 256 * g, D + 256 * g, 2 * D + 256 * g
        wqk = np.ascontiguousarray(
            np.concatenate(
                [w_qkv[:, qs : qs + GD], w_qkv[:, ks : ks + GD]], axis=1
            ).astype(ml_dtypes.bfloat16)
        )
        wv = np.ascontiguousarray(
            w_qkv[:, vs : vs + GD].astype(ml_dtypes.bfloat16))
        wo = np.ascontiguousarray(
            w_out[256 * g : 256 * g + GD, :].astype(ml_dtypes.bfloat16))
        bqk = np.concatenate(
            [b_qkv[qs : qs + GD], b_qkv[ks : ks + GD]]
        ).reshape(2 * GD, 1).astype(np.float32)
        in_maps.append(
            {"xT": xTs[b], "wqk": wqk, "wv": wv, "wo": wo, "bqk": bqk,
             "maskT": maskT, "ident": ident, "vpat": make_vpat()}
        )
    return in_maps


_NC_CACHE = {}


def get_nc(L=2048):
    if L not in _NC_CACHE:
        _NC_CACHE[L] = build_nc(L)
    return _NC_CACHE[L]


def gather(results, b_qkv, w_out, b_out, L=2048):
    fix = (np.asarray(b_qkv, np.float32)[2 * D :] @ np.asarray(w_out, np.float32)
           + np.asarray(b_out, np.float32))
    y = np.zeros((B, L, D), np.float32)
    for c in range(8):
        b = c // 4
        y[b] += results[c]["y"]
    y += fix[None, None, :]
    return y


def kernel(x, w_qkv, b_qkv, w_out, b_out):
    L = x.shape[1]
    nc = get_nc(L)
    in_maps = shard_inputs(x, w_qkv, b_qkv, w_out, L=L)
    res = run_bass_kernel_spmd(nc, in_maps, core_ids=list(range(8)))
    return gather(res.results, b_qkv, w_out, b_out, L=L)
